# revision 11
# baseline (speedup 1.0000x reference)
"""DGCNN (3x DynamicEdgeConv, kNN=20) Trainium2 Bass kernel.

Self-contained: `kernel(**inputs) -> np.ndarray` takes the full inputs from
setup_inputs() (pos [8,4096,3] + 9 weight/bias pairs) and returns [8,4096,64].

Sharding: NC=2 NeuronCores, NB=4 whole point clouds per core, weights
replicated. Few cores is deliberate: per-call dispatch overhead on this axon
client scales ~5 ms per device in the sharded call, while one cloud's device
span is only ~4 ms, so 2 cores x 4 sequential clouds beats 8 x 1.

Per-core, per-cloud, per-layer pipeline (N=4096 points, D in {3,64}, H=64):
  phase 0: negsq2 = -2*||x_j||^2 row (ACT square + PE colsum);
           U = x@(W1a-W1b)+b1 (point-major, SBUF), V = x@W1b -> DRAM [4096,64]
  per row-tile t (128 points):
    S = 4 x_i.x_j - 2 sq_j  (PE, rank-1 ones x negsq2 accumulated into the
        same PSUM tile; row-monotone == -dist)
    top-20: 3 rounds of DVE max8 / max_index / match_replace -> 24 winners
    h1 = leaky(U_i + V_j): prefill U, 20x indirect-DMA gather with CCE-add
    h1 -> PE transposes -> h1T [64, 20*128] (edge k-major)
    h2T = Prelu(W2^T@h1T + b2), h3T = Prelu(W3^T@h2T + b3)   (PE + ACT)
    out tile = max over k (DVE strided reduce) -> next layer's 2*X^T
"""
import os
import numpy as np

import concourse.bass as bass
import concourse.bacc as bacc
import concourse.mybir as mybir
import concourse.tile as tile
from concourse.masks import make_identity

F32 = mybir.dt.float32
U32 = mybir.dt.uint32
AF = mybir.ActivationFunctionType
ALU = mybir.AluOpType

B = 8                  # total point clouds
NC = int(os.environ.get("NC", "2"))     # NeuronCores used
NB = B // NC           # clouds per core
NCORES = NC
N = 4096
P = 128
NT = N // P            # 32 row tiles
K = 20
H = 64
SLOPE = 0.2
NEG = -3.0e38
SKIP_GATHER = bool(int(os.environ.get("SKIP_GATHER", "0")))
SKIP_TOPK = bool(int(os.environ.get("SKIP_TOPK", "0")))
STAGE = int(os.environ.get("STAGE", "4"))  # truncate per-tile pipeline for attribution
SIM_ACT = bool(int(os.environ.get("SIM_ACT", "0")))  # Prelu->Copy for CoreSim
NSW = int(os.environ.get("NSW", "4"))      # SWDGE queues for the edge gathers
REPEAT = int(os.environ.get("REPEAT", "1"))  # loop whole pipeline R times (timing)

_CACHE = {}


def _gather_q(nc, q, **kw):
    bi = nc.gpsimd.indirect_dma_start(**kw)
    if q:
        bi.ins.queue = f"qPoolDynamic{q}"
    return bi


def _prep_weights(nc, g, li, d_in, wn):
    """Load + derive one layer's weights into persistent SBUF tiles."""
    sb = g["sb"]
    w1_d, b1_d = wn[f"w{li+1}1"], wn[f"b{li+1}1"]
    w2_d, b2_d = wn[f"w{li+1}2"], wn[f"b{li+1}2"]
    w3_d, b3_d = wn[f"w{li+1}3"], wn[f"b{li+1}3"]
    wa = sb.tile([d_in, H], F32, tag=f"wa{li}")
    wb = sb.tile([d_in, H], F32, tag=f"wb{li}")
    nc.sync.dma_start(wa[:], w1_d[0:d_in, :])
    nc.sync.dma_start(wb[:], w1_d[d_in:2 * d_in, :])
    wd2 = sb.tile([d_in, H], F32, tag=f"wd2{li}")
    nc.vector.tensor_tensor(out=wd2[:], in0=wa[:], in1=wb[:], op=ALU.subtract)
    nc.vector.tensor_scalar_mul(wd2[:], wd2[:], 0.5)
    wb2 = sb.tile([d_in, H], F32, tag=f"wb2{li}")
    nc.vector.tensor_scalar_mul(wb2[:], wb[:], 0.5)
    w2 = sb.tile([H, H], F32, tag=f"w2{li}")
    nc.sync.dma_start(w2[:], w2_d[:])
    w3 = sb.tile([H, H], F32, tag=f"w3{li}")
    nc.sync.dma_start(w3[:], w3_d[:])
    b1r = sb.tile([1, H], F32, tag=f"b1r{li}")
    nc.sync.dma_start(b1r[:], b1_d[:].unsqueeze(0))
    b2c = sb.tile([H, 1], F32, tag=f"b2c{li}")
    nc.sync.dma_start(b2c[:], b2_d[:].unsqueeze(1))
    b3c = sb.tile([H, 1], F32, tag=f"b3c{li}")
    nc.sync.dma_start(b3c[:], b3_d[:].unsqueeze(1))
    return dict(wd2=wd2, wb2=wb2, w2=w2, w3=w3, b1r=b1r, b2c=b2c, b3c=b3c)


def _build_layer(nc, g, li, d_in, W, x2t, x2t_next, v_d, out_view=None):
    """Emit one EdgeConv layer for one cloud. x2t holds 2*X^T (rows 0..d_in-1).
    Writes 2*X_next^T into x2t_next, or the output tile rows into out_view."""
    pp_s, pp_tp, pp_h = g["pp_s"], g["pp_tp"], g["pp_h"]
    ident, ones1, alpha64, alpha128 = (
        g["ident"], g["ones1"], g["alpha64"], g["alpha128"])
    wd2, wb2, w2, w3 = W["wd2"], W["wb2"], W["w2"], W["w3"]
    b1r, b2c, b3c = W["b1r"], W["b2c"], W["b3c"]

    # ---- phase 0: negsq2 = -2*sq (colsum(X2T^2) = 4 sq, scaled by -0.5) ----
    xsq = g["s_pool"].tile([P, N], F32, tag="s")   # borrow an S buffer
    nc.scalar.activation(out=xsq[0:d_in, :], in_=x2t[0:d_in, :], func=AF.Square)
    negsq2 = g["nq_pool"].tile([1, N], F32, tag="negsq")
    for c in range(N // 512):
        ps = pp_tp.tile([P, 512], F32, tag="tp")
        nc.tensor.matmul(out=ps[0:1, :], lhsT=g["onescol"][0:d_in, :],
                         rhs=xsq[0:d_in, c * 512:(c + 1) * 512],
                         start=True, stop=True)
        nc.scalar.activation(out=negsq2[:, c * 512:(c + 1) * 512], in_=ps[0:1, :],
                             func=AF.Copy, scale=-0.5)

    # ---- phase 0: U (point-major SBUF) and V (point-major DRAM) ----
    u_sb = g["u_pool"].tile([P, NT * H], F32, tag="u")
    vbuf = g["v_pool"].tile([P, NT * H], F32, tag="vbuf")
    for t in range(NT):
        lhs = x2t[0:d_in, t * P:(t + 1) * P]
        pu = pp_tp.tile([P, 512], F32, tag="tp")
        nc.tensor.matmul(out=pu[:, 0:H], lhsT=lhs, rhs=wd2[:], start=True, stop=False)
        nc.tensor.matmul(out=pu[:, 0:H], lhsT=ones1[:, 0:P], rhs=b1r[:],
                         start=False, stop=True)
        nc.tensor.matmul(out=pu[:, H:2 * H], lhsT=lhs, rhs=wb2[:], start=True, stop=True)
        nc.scalar.copy(out=u_sb[:, t * H:(t + 1) * H], in_=pu[:, 0:H])
        nc.scalar.copy(out=vbuf[:, t * H:(t + 1) * H], in_=pu[:, H:2 * H])
    nc.sync.dma_start(v_d[:].rearrange("(t p) f -> p t f", p=P),
                      vbuf[:].rearrange("p (t f) -> p t f", f=H))

    if STAGE < 4:
        if out_view is None:
            nc.vector.memset(x2t_next[0:H, :], 0.5)
        else:
            nc.vector.memset(out_view[:], 0.5)
    if STAGE < 1:
        return
    # ---- per row-tile ----
    for t in range(NT):
        # distance tile S [128, N] (bigger = closer); -2sq_j folded in as a
        # rank-1 accumulate (ones x negsq2) into the same PSUM tile
        s_sb = g["s_pool"].tile([P, N], F32, tag="s")
        for c in range(N // 1024):
            ps = pp_s.tile([P, 1024], F32, tag="dist")
            for hh in range(2):
                cc = c * 1024 + hh * 512
                nc.tensor.matmul(out=ps[:, hh * 512:(hh + 1) * 512],
                                 lhsT=x2t[0:d_in, t * P:(t + 1) * P],
                                 rhs=x2t[0:d_in, cc:cc + 512],
                                 start=True, stop=False)
                nc.tensor.matmul(out=ps[:, hh * 512:(hh + 1) * 512],
                                 lhsT=ones1[:, 0:P],
                                 rhs=negsq2[:, cc:cc + 512],
                                 start=False, stop=True)
            nc.scalar.copy(out=s_sb[:, c * 1024:(c + 1) * 1024], in_=ps[:])

        # exact top-24 of each row with in-place knockout (8 wide DVE insts):
        # max8 -> indices -> replace-with-NEG, three rounds. match_replace
        # only zaps already-extracted values, so later max_index positions
        # in the modified S equal positions in the original.
        winners = g["k_pool"].tile([P, 24], F32, tag="win")
        idxt = g["k_pool"].tile([P, 24], U32, tag="idxt")
        if SKIP_TOPK:
            nc.vector.memset(winners[:], 0.0)
            nc.vector.memset(idxt[:], 0)
        else:
            for r in range(3):
                nc.vector.max(out=winners[:, r * 8:(r + 1) * 8], in_=s_sb[:])
                nc.vector.max_index(out=idxt[:, r * 8:(r + 1) * 8],
                                    in_max=winners[:, r * 8:(r + 1) * 8],
                                    in_values=s_sb[:])
                if r < 2:
                    nc.vector.match_replace(out=s_sb[:],
                                            in_to_replace=winners[:, r * 8:(r + 1) * 8],
                                            in_values=s_sb[:], imm_value=NEG)

        if STAGE < 2:
            continue
        # edge features: h1 = leaky(U_i + V_j)
        h1 = g["h1_pool"].tile([P, K * H], F32, tag="h1")
        h1v = h1[:].rearrange("p (k f) -> p k f", k=K)
        u_bc = u_sb[:, t * H:(t + 1) * H].unsqueeze(1).to_broadcast([P, K, H])
        nc.scalar.copy(out=h1v, in_=u_bc)
        for k in (range(0) if SKIP_GATHER else range(K)):
            _gather_q(nc, k % NSW,
                      out=h1[:, k * H:(k + 1) * H], out_offset=None,
                      in_=v_d[:],
                      in_offset=bass.IndirectOffsetOnAxis(ap=idxt[:, k:k + 1], axis=0),
                      compute_op=ALU.add)
        if SIM_ACT:
            nc.scalar.activation(out=h1[:], in_=h1[:], func=AF.Copy)
        else:
            nc.scalar.activation(out=h1[:], in_=h1[:], func=AF.Prelu, alpha=alpha128[:])

        if STAGE < 3:
            continue
        # transpose to edge-major h1T [64, k*128+i]
        h1t = g["ht_pool"].tile([H, K * P], F32, tag="ht")
        for kc in range(5):
            pt = pp_tp.tile([P, 512], F32, tag="tp")
            for j in range(4):
                k = kc * 4 + j
                nc.tensor.transpose(out=pt[0:H, j * P:(j + 1) * P],
                                    in_=h1[:, k * H:(k + 1) * H], identity=ident[:])
            nc.scalar.copy(out=h1t[:, kc * 512:(kc + 1) * 512], in_=pt[0:H, :])

        if STAGE < 4:
            continue
        # MLP layers 2, 3 (feature-major, edges on the free axis)
        h2t = g["ht_pool"].tile([H, K * P], F32, tag="ht")
        for e in range(5):
            ph = pp_h.tile([H, 512], F32, tag="h")
            nc.tensor.matmul(out=ph[:], lhsT=w2[:],
                             rhs=h1t[:, e * 512:(e + 1) * 512], start=True, stop=True)
            nc.scalar.activation(out=h2t[:, e * 512:(e + 1) * 512], in_=ph[:],
                                 func=AF.Copy if SIM_ACT else AF.Prelu,
                                 bias=0.0 if SIM_ACT else b2c[:],
                                 alpha=0.0 if SIM_ACT else alpha64[:])
        h3t = g["ht_pool"].tile([H, K * P], F32, tag="ht")
        for e in range(5):
            ph = pp_h.tile([H, 512], F32, tag="h")
            nc.tensor.matmul(out=ph[:], lhsT=w3[:],
                             rhs=h2t[:, e * 512:(e + 1) * 512], start=True, stop=True)
            nc.scalar.activation(out=h3t[:, e * 512:(e + 1) * 512], in_=ph[:],
                                 func=AF.Copy if SIM_ACT else AF.Prelu,
                                 bias=0.0 if SIM_ACT else b3c[:],
                                 alpha=0.0 if SIM_ACT else alpha64[:])

        # aggregate: max over k (innermost stride-128 axis)
        ftile = g["f_pool"].tile([H, P], F32, tag="f")
        nc.vector.tensor_reduce(out=ftile[:],
                                in_=h3t[:].rearrange("h (k i) -> h i k", k=K),
                                axis=mybir.AxisListType.X, op=ALU.max)
        if out_view is None:
            nc.scalar.mul(out=x2t_next[0:H, t * P:(t + 1) * P], in_=ftile[:], mul=2.0)
        else:
            po = pp_tp.tile([P, 512], F32, tag="tp")
            nc.tensor.transpose(out=po[:, 0:H], in_=ftile[:], identity=ident[0:H, 0:H])
            nc.scalar.copy(out=out_view[:, t * H:(t + 1) * H], in_=po[:, 0:H])


# weight-blob layout: per layer w1, w2, w3, b1, b2, b3 (fp32 elements).
# Packing all 18 weight tensors into ONE device input matters: each extra
# input tensor costs ~1.5 ms of per-call host/axon binding overhead.
def _blob_layout():
    off, lay = 0, []
    for li in range(3):
        d2 = 6 if li == 0 else 128
        ent = {}
        for nm, shp in ((f"w{li+1}1", (d2, H)), (f"w{li+1}2", (H, H)),
                        (f"w{li+1}3", (H, H)), (f"b{li+1}1", (H,)),
                        (f"b{li+1}2", (H,)), (f"b{li+1}3", (H,))):
            n = int(np.prod(shp))
            ent[nm] = (off, shp)
            off += n
        lay.append(ent)
    return lay, off


def build():
    nc = bacc.Bacc("TRN2", target_bir_lowering=False, debug=False,
                   num_swdge_queues=NSW)
    pos_d = nc.dram_tensor("pos", [NB * N, 3], F32, kind="ExternalInput")
    lay, tot = _blob_layout()
    wblob_d = nc.dram_tensor("wblob", [tot], F32, kind="ExternalInput")
    wnames = {}
    for li in range(3):
        for nm, (off, shp) in lay[li].items():
            n = int(np.prod(shp))
            v = wblob_d[off:off + n]
            if len(shp) == 2:
                v = v.rearrange("(r c) -> r c", c=shp[1])
            wnames[nm] = v
    out_d = nc.dram_tensor("out", [NB * N, H], F32, kind="ExternalOutput")
    # per-(layer, cloud-parity) V tables so adjacent clouds never share one
    v_ds = [[nc.dram_tensor(f"vtab{li}_{p}", [N, H], F32) for p in range(2)]
            for li in range(3)]

    with tile.TileContext(nc) as tc:
        with tc.tile_pool(name="sb", bufs=1) as sb, \
             tc.tile_pool(name="xt_pool", bufs=3) as xt_pool, \
             tc.tile_pool(name="s_pool", bufs=2) as s_pool, \
             tc.tile_pool(name="nq_pool", bufs=2) as nq_pool, \
             tc.tile_pool(name="u_pool", bufs=2) as u_pool, \
             tc.tile_pool(name="v_pool", bufs=1) as v_pool, \
             tc.tile_pool(name="o_pool", bufs=1) as o_pool, \
             tc.tile_pool(name="k_pool", bufs=2) as k_pool, \
             tc.tile_pool(name="h1_pool", bufs=2) as h1_pool, \
             tc.tile_pool(name="ht_pool", bufs=4) as ht_pool, \
             tc.tile_pool(name="f_pool", bufs=2) as f_pool, \
             tc.tile_pool(name="pp_s", bufs=2, space="PSUM") as pp_s, \
             tc.tile_pool(name="pp_tp", bufs=2, space="PSUM") as pp_tp, \
             tc.tile_pool(name="pp_h", bufs=2, space="PSUM") as pp_h:

            g = dict(sb=sb, s_pool=s_pool, nq_pool=nq_pool, u_pool=u_pool,
                     v_pool=v_pool, k_pool=k_pool, h1_pool=h1_pool,
                     ht_pool=ht_pool, f_pool=f_pool,
                     pp_s=pp_s, pp_tp=pp_tp, pp_h=pp_h)

            ident = sb.tile([P, P], F32, tag="ident")
            make_identity(nc, ident)
            g["ident"] = ident
            ones1 = sb.tile([1, P], F32, tag="ones1")
            nc.vector.memset(ones1[:], 1.0)
            g["ones1"] = ones1
            onescol = sb.tile([H, 1], F32, tag="onescol")
            nc.vector.memset(onescol[:], 1.0)
            g["onescol"] = onescol
            alpha64 = sb.tile([H, 1], F32, tag="alpha64")
            nc.vector.memset(alpha64[:], SLOPE)
            g["alpha64"] = alpha64
            alpha128 = sb.tile([P, 1], F32, tag="alpha128")
            nc.vector.memset(alpha128[:], SLOPE)
            g["alpha128"] = alpha128

            Ws = [_prep_weights(nc, g, li, 3 if li == 0 else H, wnames)
                  for li in range(3)]

            for rep in range(REPEAT):
                for c in range(NB):
                    # load cloud c's pos -> 2*X^T (rows 0..2)
                    x2t_a = xt_pool.tile([H, N], F32, tag="x2t")
                    xsb = u_pool.tile([P, NT * H], F32, tag="u")  # borrow
                    nc.sync.dma_start(
                        xsb[:, 0:NT * 3].rearrange("p (t d) -> p t d", d=3),
                        pos_d[c * N:(c + 1) * N].rearrange("(t p) d -> p t d", p=P))
                    for t in range(NT):
                        pt = pp_tp.tile([P, 512], F32, tag="tp")
                        nc.tensor.transpose(out=pt[0:3, 0:P],
                                            in_=xsb[:, t * 3:(t + 1) * 3],
                                            identity=ident[:])
                        nc.scalar.mul(out=x2t_a[0:3, t * P:(t + 1) * P],
                                      in_=pt[0:3, 0:P], mul=2.0)

                    x2t_b = xt_pool.tile([H, N], F32, tag="x2t")
                    _build_layer(nc, g, 0, 3, Ws[0], x2t_a, x2t_b, v_ds[0][c % 2])
                    x2t_c = xt_pool.tile([H, N], F32, tag="x2t")
                    _build_layer(nc, g, 1, H, Ws[1], x2t_b, x2t_c, v_ds[1][c % 2])
                    obuf = o_pool.tile([P, NT * H], F32, tag="obuf")
                    _build_layer(nc, g, 2, H, Ws[2], x2t_c, None, v_ds[2][c % 2],
                                 out_view=obuf)
                    nc.sync.dma_start(
                        out_d[c * N:(c + 1) * N].rearrange("(t p) f -> p t f", p=P),
                        obuf[:].rearrange("p (t f) -> p t f", f=H))
    nc.finalize()
    return nc


def pack_wblob(inputs):
    lay, tot = _blob_layout()
    blob = np.empty(tot, np.float32)
    for li in range(3):
        for nm, (off, shp) in lay[li].items():
            a = np.asarray(inputs[nm], dtype=np.float32).reshape(-1)
            blob[off:off + a.size] = a
    return blob


def make_in_maps(inputs):
    pos = np.ascontiguousarray(np.asarray(inputs["pos"], dtype=np.float32))
    blob = pack_wblob(inputs)
    pc = pos.reshape(NC, NB * N, 3)
    return [{"pos": pc[c], "wblob": blob} for c in range(NC)]


def _make_runner(nc):
    """Cached jitted NEFF executor: warm kernel() calls skip retracing.
    The weight blob is passed replicated (one host copy, not an NC-x concat)."""
    import jax
    from jax.sharding import Mesh, PartitionSpec
    from jax.experimental.shard_map import shard_map
    from concourse.bass2jax import (_bass_exec_p, install_neuronx_cc_hook,
                                    partition_id_tensor)
    install_neuronx_cc_hook()
    partition_name = nc.partition_id_tensor.name if nc.partition_id_tensor else None
    in_names, out_names, out_avals, zero_shapes = [], [], [], []
    for alloc in nc.m.functions[0].allocations:
        if not isinstance(alloc, mybir.MemoryLocationSet):
            continue
        name = alloc.memorylocations[0].name
        if alloc.kind == "ExternalInput":
            if name != partition_name:
                in_names.append(name)
        elif alloc.kind == "ExternalOutput":
            out_names.append(name)
            shape = tuple(alloc.tensor_shape)
            dtype = mybir.dt.np(alloc.dtype)
            out_avals.append(jax.core.ShapedArray(shape, dtype))
            zero_shapes.append((shape, dtype))
    n_params = len(in_names)
    n_outs = len(out_avals)
    in_names_all = list(in_names) + out_names
    if partition_name is not None:
        in_names_all.append(partition_name)

    def _body(*args):
        operands = list(args)
        if partition_name is not None:
            operands.append(partition_id_tensor())
        return tuple(_bass_exec_p.bind(
            *operands, out_avals=tuple(out_avals),
            in_names=tuple(in_names_all), out_names=tuple(out_names),
            lowering_input_output_aliases=(),
            sim_require_finite=True, sim_require_nnan=True, nc=nc))

    devices = jax.devices()[:NC]
    mesh = Mesh(np.asarray(devices), ("core",))
    rep_names = {"wblob"} & set(in_names)
    in_specs = tuple(
        PartitionSpec() if nm in rep_names else PartitionSpec("core")
        for nm in in_names
    ) + (PartitionSpec("core"),) * n_outs
    sharded = jax.jit(
        shard_map(_body, mesh=mesh, in_specs=in_specs,
                  out_specs=(PartitionSpec("core"),) * len(out_names),
                  check_rep=False),
        donate_argnums=tuple(range(n_params, n_params + n_outs)),
        keep_unused=True,
    )

    # Output buffers are pure scratch (the kernel fully overwrites out_d), so
    # they are chained across calls via donation: the first call ships zeros,
    # every later call re-donates the previous device-resident buffers. This
    # removes an 8 MB host->device upload (~80 ms of axon RPC) per warm call.
    state = {"outs": None}

    def submit(in_maps):
        """Upload fresh inputs, launch, return device output handles."""
        per_core = [[np.asarray(m[name]) for name in in_names] for m in in_maps]
        args_in = [
            per_core[0][i] if in_names[i] in rep_names else
            np.concatenate([per_core[c][i] for c in range(NC)], axis=0)
            for i in range(n_params)
        ]
        outs = state["outs"]
        if outs is None:
            outs = [np.zeros((NC * sh[0], *sh[1:]), dt) for sh, dt in zero_shapes]
        out_arrs = list(sharded(*args_in, *outs))
        state["outs"] = out_arrs
        jax.block_until_ready(out_arrs)
        return out_arrs

    def run(in_maps):
        out_arrs = submit(in_maps)
        return [
            {name: np.asarray(out_arrs[i]).reshape(NC, *out_avals[i].shape)[c]
             for i, name in enumerate(out_names)}
            for c in range(NC)
        ]

    run.submit = submit
    return run


def kernel(**inputs):
    if "nc" not in _CACHE:
        _CACHE["nc"] = build()
        _CACHE["run"] = _make_runner(_CACHE["nc"])
    in_maps = make_in_maps(inputs)
    results = _CACHE["run"](in_maps)
    out = np.concatenate([results[c]["out"] for c in range(NC)], axis=0)
    return out.reshape(B, N, H)


if __name__ == "__main__":
    rng = np.random.default_rng(0)
    fake = {"pos": rng.standard_normal((B, N, 3)).astype(np.float32)}
    for pfx in ("1", "2", "3"):
        d2 = 6 if pfx == "1" else 128
        fake[f"w{pfx}1"] = rng.standard_normal((d2, H)).astype(np.float32) * 0.2
        fake[f"w{pfx}2"] = rng.standard_normal((H, H)).astype(np.float32) * 0.12
        fake[f"w{pfx}3"] = rng.standard_normal((H, H)).astype(np.float32) * 0.12
        for j in ("1", "2", "3"):
            fake[f"b{pfx}{j}"] = np.zeros(H, np.float32)
    o = kernel(**fake)
    print("out", o.shape, o.dtype, float(np.abs(o).max()))


# revision 26
# speedup vs baseline: 1.3592x; 1.3592x over previous
"""DGCNN (3x DynamicEdgeConv, kNN=20) Trainium2 Bass kernel.

Self-contained: `kernel(**inputs) -> np.ndarray` takes the full inputs from
setup_inputs() (pos [8,4096,3] + 9 weight/bias pairs) and returns [8,4096,64].

Sharding: NC=2 NeuronCores, NB=4 whole point clouds per core, weights
replicated. Few cores is deliberate: per-call dispatch overhead on this axon
client scales ~5 ms per device in the sharded call, while one cloud's device
span is only ~4 ms, so 2 cores x 4 sequential clouds beats 8 x 1.

Per-core, per-cloud, per-layer pipeline (N=4096 points, D in {3,64}, H=64):
  phase 0: negsq2 = -2*||x_j||^2 row (ACT square + PE colsum);
           U = x@(W1a-W1b)+b1 (point-major, SBUF), V = x@W1b -> DRAM [4096,64]
  per row-tile t (128 points):
    S = 4 x_i.x_j - 2 sq_j  (PE, rank-1 ones x negsq2 accumulated into the
        same PSUM tile; row-monotone == -dist)
    top-20: 3 rounds of DVE max8 / max_index / match_replace -> 24 winners
    h1 = leaky(U_i + V_j): prefill U, 20x indirect-DMA gather with CCE-add
    h1 -> PE transposes -> h1T [64, 20*128] (edge k-major)
    h2T = Prelu(W2^T@h1T + b2), h3T = Prelu(W3^T@h2T + b3)   (PE + ACT)
    out tile = max over k (DVE strided reduce) -> next layer's 2*X^T
"""
import os
import numpy as np

import concourse.bass as bass
import concourse.bacc as bacc
import concourse.mybir as mybir
import concourse.tile as tile
from concourse.masks import make_identity

F32 = mybir.dt.float32
U32 = mybir.dt.uint32
AF = mybir.ActivationFunctionType
ALU = mybir.AluOpType

B = 8                  # total point clouds
NC = int(os.environ.get("NC", "2"))     # NeuronCores used
NB = B // NC           # clouds per core
NCORES = NC
N = 4096
P = 128
NT = N // P            # 32 row tiles
K = 20
H = 64
SLOPE = 0.2
NEG = -3.0e38
SKIP_GATHER = bool(int(os.environ.get("SKIP_GATHER", "0")))
SKIP_TOPK = bool(int(os.environ.get("SKIP_TOPK", "0")))
STAGE = int(os.environ.get("STAGE", "4"))  # truncate per-tile pipeline for attribution
SIM_ACT = bool(int(os.environ.get("SIM_ACT", "0")))  # Prelu->Copy for CoreSim
NSW = int(os.environ.get("NSW", "1"))      # SWDGE queues (extra queues cost ~10 ms/call each in per-call runtime setup -- keep 1)
REPEAT = int(os.environ.get("REPEAT", "1"))  # loop whole pipeline R times (timing)

_CACHE = {}


def _gather_q(nc, q, **kw):
    bi = nc.gpsimd.indirect_dma_start(**kw)
    if q:
        bi.ins.queue = f"qPoolDynamic{q}"
    return bi


def _prep_weights(nc, g, li, d_in, wn):
    """Load + derive one layer's weights into persistent SBUF tiles."""
    sb = g["sb"]
    w1_d, b1_d = wn[f"w{li+1}1"], wn[f"b{li+1}1"]
    w2_d, b2_d = wn[f"w{li+1}2"], wn[f"b{li+1}2"]
    w3_d, b3_d = wn[f"w{li+1}3"], wn[f"b{li+1}3"]
    wa = sb.tile([d_in, H], F32, tag=f"wa{li}")
    wb = sb.tile([d_in, H], F32, tag=f"wb{li}")
    nc.sync.dma_start(wa[:], w1_d[0:d_in, :])
    nc.sync.dma_start(wb[:], w1_d[d_in:2 * d_in, :])
    wd2 = sb.tile([d_in, H], F32, tag=f"wd2{li}")
    nc.vector.tensor_tensor(out=wd2[:], in0=wa[:], in1=wb[:], op=ALU.subtract)
    nc.vector.tensor_scalar_mul(wd2[:], wd2[:], 0.5)
    wb2 = sb.tile([d_in, H], F32, tag=f"wb2{li}")
    nc.vector.tensor_scalar_mul(wb2[:], wb[:], 0.5)
    w2 = sb.tile([H, H], F32, tag=f"w2{li}")
    nc.sync.dma_start(w2[:], w2_d[:])
    w3 = sb.tile([H, H], F32, tag=f"w3{li}")
    nc.sync.dma_start(w3[:], w3_d[:])
    b1r = sb.tile([1, H], F32, tag=f"b1r{li}")
    nc.sync.dma_start(b1r[:], b1_d[:].unsqueeze(0))
    b2c = sb.tile([H, 1], F32, tag=f"b2c{li}")
    nc.sync.dma_start(b2c[:], b2_d[:].unsqueeze(1))
    b3c = sb.tile([H, 1], F32, tag=f"b3c{li}")
    nc.sync.dma_start(b3c[:], b3_d[:].unsqueeze(1))
    return dict(wd2=wd2, wb2=wb2, w2=w2, w3=w3, b1r=b1r, b2c=b2c, b3c=b3c)


def _build_layer(nc, g, li, d_in, W, x2t, x2t_next, v_d, out_view=None):
    """Emit one EdgeConv layer for one cloud. x2t holds 2*X^T (rows 0..d_in-1).
    Writes 2*X_next^T into x2t_next, or the output tile rows into out_view."""
    pp_s, pp_tp, pp_h = g["pp_s"], g["pp_tp"], g["pp_h"]
    ident, ones1, alpha64, alpha128 = (
        g["ident"], g["ones1"], g["alpha64"], g["alpha128"])
    wd2, wb2, w2, w3 = W["wd2"], W["wb2"], W["w2"], W["w3"]
    b1r, b2c, b3c = W["b1r"], W["b2c"], W["b3c"]

    # ---- phase 0: negsq2 = -2*sq (colsum(X2T^2) = 4 sq, scaled by -0.5) ----
    xsq = g["s_pool"].tile([P, N], F32, tag="s")   # borrow an S buffer
    nc.scalar.activation(out=xsq[0:d_in, :], in_=x2t[0:d_in, :], func=AF.Square)
    # -2sq row, then replicate across partitions (PE rank-1 ones x row), so
    # the per-tile correction is a single wide GPSIMD add: putting it on the
    # PE as per-tile rank-1 accumulates would double the S matmul cost (fp32
    # matmul cost is the 512-col stream length regardless of contraction)
    negsq2 = g["nq_pool"].tile([1, N], F32, tag="negsq")
    negsq_rep = g["nr_pool"].tile([P, N], F32, tag="negsqrep")
    for c in range(N // 512):
        ps = pp_tp.tile([P, 512], F32, tag="tp")
        nc.tensor.matmul(out=ps[0:1, :], lhsT=g["onescol"][0:d_in, :],
                         rhs=xsq[0:d_in, c * 512:(c + 1) * 512],
                         start=True, stop=True)
        nc.scalar.activation(out=negsq2[:, c * 512:(c + 1) * 512],
                             in_=ps[0:1, :], func=AF.Copy, scale=-0.5)
    for c in range(N // 512):
        ps = pp_tp.tile([P, 512], F32, tag="tp")
        nc.tensor.matmul(out=ps[:], lhsT=ones1[:, 0:P],
                         rhs=negsq2[:, c * 512:(c + 1) * 512],
                         start=True, stop=True)
        nc.scalar.copy(out=negsq_rep[:, c * 512:(c + 1) * 512], in_=ps[:])

    # ---- phase 0: U (point-major SBUF) and V (point-major DRAM) ----
    # 4 row tiles per PSUM tile; shared lhsT loads, then 4 bias matmuls with
    # a single ones-vector load; one wide ACT copy per quad
    u_sb = g["u_pool"].tile([P, NT * H], F32, tag="u")
    vbuf = g["v_pool"].tile([P, NT * H], F32, tag="vbuf")
    for t in range(NT):
        lhs = x2t[0:d_in, t * P:(t + 1) * P]
        pu = pp_tp.tile([P, 512], F32, tag="tp")
        nc.tensor.matmul(out=pu[:, 0:H], lhsT=lhs, rhs=wd2[:], start=True, stop=False)
        nc.tensor.matmul(out=pu[:, 0:H], lhsT=ones1[:, 0:P], rhs=b1r[:],
                         start=False, stop=True)
        nc.tensor.matmul(out=pu[:, H:2 * H], lhsT=lhs, rhs=wb2[:], start=True, stop=True)
        nc.scalar.copy(out=u_sb[:, t * H:(t + 1) * H], in_=pu[:, 0:H])
        nc.scalar.copy(out=vbuf[:, t * H:(t + 1) * H], in_=pu[:, H:2 * H])
    nc.sync.dma_start(v_d[:].rearrange("(t p) f -> p t f", p=P),
                      vbuf[:].rearrange("p (t f) -> p t f", f=H))

    if STAGE < 4:
        if out_view is None:
            nc.vector.memset(x2t_next[0:H, :], 0.5)
        else:
            nc.vector.memset(out_view[:], 0.5)
    if STAGE < 1:
        return

    # ---- per row-tile, software-pipelined 3 deep so no engine's in-order
    # stream stalls: S for tile t+1 is produced while DVE runs topk(t), and
    # the gather-dependent tail of tile t-1 (Prelu/transpose/MLP/reduce) runs
    # behind, after its gather DMAs have long landed. Without this the DVE
    # stream [topk(t), reduce(t)] idles ~35us/tile waiting on the MLP chain.
    st = {}  # per-tile live handles

    def emit_S(t):
        # distance tile S [128, N] (bigger = closer): PE with one stationary
        # load per tile, then a single wide GPSIMD add of -2sq_j
        s_sb = g["s_pool"].tile([P, N], F32, tag="s")
        for c in range(N // 1024):
            ps = pp_s.tile([P, 1024], F32, tag="dist")
            for hh in range(2):
                cc = c * 1024 + hh * 512
                nc.tensor.matmul(out=ps[:, hh * 512:(hh + 1) * 512],
                                 lhsT=x2t[0:d_in, t * P:(t + 1) * P],
                                 rhs=x2t[0:d_in, cc:cc + 512],
                                 start=True, stop=True)
            nc.scalar.copy(out=s_sb[:, c * 1024:(c + 1) * 1024], in_=ps[:])
        nc.gpsimd.tensor_tensor(out=s_sb[:], in0=s_sb[:], in1=negsq_rep[:],
                                op=ALU.add)
        st[t] = {"s": s_sb}

    def emit_topk_gather(t):
        s_sb = st[t]["s"]
        # exact top-24 of each row with in-place knockout (8 wide DVE insts):
        # max8 -> indices -> replace-with-NEG, three rounds. match_replace
        # only zaps already-extracted values, so later max_index positions
        # in the modified S equal positions in the original.
        winners = g["k_pool"].tile([P, 24], F32, tag="win")
        idxt = g["k_pool"].tile([P, 24], U32, tag="idxt")
        if SKIP_TOPK:
            nc.vector.memset(winners[:], 0.0)
            nc.vector.memset(idxt[:], 0)
        else:
            for r in range(3):
                nc.vector.max(out=winners[:, r * 8:(r + 1) * 8], in_=s_sb[:])
                nc.vector.max_index(out=idxt[:, r * 8:(r + 1) * 8],
                                    in_max=winners[:, r * 8:(r + 1) * 8],
                                    in_values=s_sb[:])
                if r < 2:
                    nc.vector.match_replace(out=s_sb[:],
                                            in_to_replace=winners[:, r * 8:(r + 1) * 8],
                                            in_values=s_sb[:], imm_value=NEG)
        if STAGE < 2:
            return
        # edge features: h1 = leaky(U_i + V_j) -- prefill U, then CCE-add
        # gathers of V_j; the Prelu happens in the delayed tail
        h1 = g["h1_pool"].tile([P, K * H], F32, tag="h1")
        h1v = h1[:].rearrange("p (k f) -> p k f", k=K)
        u_bc = u_sb[:, t * H:(t + 1) * H].unsqueeze(1).to_broadcast([P, K, H])
        nc.scalar.copy(out=h1v, in_=u_bc)
        for k in (range(0) if SKIP_GATHER else range(K)):
            _gather_q(nc, k % NSW,
                      out=h1[:, k * H:(k + 1) * H], out_offset=None,
                      in_=v_d[:],
                      in_offset=bass.IndirectOffsetOnAxis(ap=idxt[:, k:k + 1], axis=0),
                      compute_op=ALU.add)
        st[t]["h1"] = h1

    def emit_tail(t):
        if STAGE < 2:
            st.pop(t, None)
            return
        h1 = st[t]["h1"]
        if SIM_ACT:
            nc.scalar.activation(out=h1[:], in_=h1[:], func=AF.Copy)
        else:
            nc.scalar.activation(out=h1[:], in_=h1[:], func=AF.Prelu, alpha=alpha128[:])
        if STAGE < 3:
            st.pop(t, None)
            return
        # transpose to edge-major h1T [64, k*128+i]
        h1t = g["ht_pool"].tile([H, K * P], F32, tag="ht")
        for kc in range(5):
            pt = pp_tp.tile([P, 512], F32, tag="tp")
            for j in range(4):
                k = kc * 4 + j
                nc.tensor.transpose(out=pt[0:H, j * P:(j + 1) * P],
                                    in_=h1[:, k * H:(k + 1) * H], identity=ident[:])
            nc.scalar.copy(out=h1t[:, kc * 512:(kc + 1) * 512], in_=pt[0:H, :])
        if STAGE < 4:
            st.pop(t, None)
            return
        # MLP layers 2, 3 (feature-major, edges on the free axis)
        h2t = g["ht_pool"].tile([H, K * P], F32, tag="ht")
        for e in range(5):
            ph = pp_h.tile([H, 512], F32, tag="h")
            nc.tensor.matmul(out=ph[:], lhsT=w2[:],
                             rhs=h1t[:, e * 512:(e + 1) * 512], start=True, stop=True)
            nc.scalar.activation(out=h2t[:, e * 512:(e + 1) * 512], in_=ph[:],
                                 func=AF.Copy if SIM_ACT else AF.Prelu,
                                 bias=0.0 if SIM_ACT else b2c[:],
                                 alpha=0.0 if SIM_ACT else alpha64[:])
        h3t = g["ht_pool"].tile([H, K * P], F32, tag="ht")
        for e in range(5):
            ph = pp_h.tile([H, 512], F32, tag="h")
            nc.tensor.matmul(out=ph[:], lhsT=w3[:],
                             rhs=h2t[:, e * 512:(e + 1) * 512], start=True, stop=True)
            nc.scalar.activation(out=h3t[:, e * 512:(e + 1) * 512], in_=ph[:],
                                 func=AF.Copy if SIM_ACT else AF.Prelu,
                                 bias=0.0 if SIM_ACT else b3c[:],
                                 alpha=0.0 if SIM_ACT else alpha64[:])
        # aggregate: max over k (innermost stride-128 axis)
        ftile = g["f_pool"].tile([H, P], F32, tag="f")
        nc.vector.tensor_reduce(out=ftile[:],
                                in_=h3t[:].rearrange("h (k i) -> h i k", k=K),
                                axis=mybir.AxisListType.X, op=ALU.max)
        if out_view is None:
            nc.scalar.mul(out=x2t_next[0:H, t * P:(t + 1) * P], in_=ftile[:], mul=2.0)
        else:
            po = pp_tp.tile([P, 512], F32, tag="tp")
            nc.tensor.transpose(out=po[:, 0:H], in_=ftile[:], identity=ident[0:H, 0:H])
            nc.scalar.copy(out=out_view[:, t * H:(t + 1) * H], in_=po[:, 0:H])
        st.pop(t, None)

    emit_S(0)
    for t in range(NT):
        if t + 1 < NT:
            emit_S(t + 1)
        emit_topk_gather(t)
        if t > 0:
            emit_tail(t - 1)
    emit_tail(NT - 1)


# weight-blob layout: per layer w1, w2, w3, b1, b2, b3 (fp32 elements).
# Packing all 18 weight tensors into ONE device input matters: each extra
# input tensor costs ~1.5 ms of per-call host/axon binding overhead.
def _blob_layout():
    off, lay = 0, []
    for li in range(3):
        d2 = 6 if li == 0 else 128
        ent = {}
        for nm, shp in ((f"w{li+1}1", (d2, H)), (f"w{li+1}2", (H, H)),
                        (f"w{li+1}3", (H, H)), (f"b{li+1}1", (H,)),
                        (f"b{li+1}2", (H,)), (f"b{li+1}3", (H,))):
            n = int(np.prod(shp))
            ent[nm] = (off, shp)
            off += n
        lay.append(ent)
    return lay, off


def build():
    nc = bacc.Bacc("TRN2", target_bir_lowering=False, debug=False,
                   num_swdge_queues=NSW)
    pos_d = nc.dram_tensor("pos", [NB * N, 3], F32, kind="ExternalInput")
    lay, tot = _blob_layout()
    wblob_d = nc.dram_tensor("wblob", [tot], F32, kind="ExternalInput")
    wnames = {}
    for li in range(3):
        for nm, (off, shp) in lay[li].items():
            n = int(np.prod(shp))
            v = wblob_d[off:off + n]
            if len(shp) == 2:
                v = v.rearrange("(r c) -> r c", c=shp[1])
            wnames[nm] = v
    out_d = nc.dram_tensor("out", [NB * N, H], F32, kind="ExternalOutput")
    # per-(layer, cloud-parity) V tables so adjacent clouds never share one
    v_ds = [[nc.dram_tensor(f"vtab{li}_{p}", [N, H], F32) for p in range(2)]
            for li in range(3)]

    with tile.TileContext(nc) as tc:
        with tc.tile_pool(name="sb", bufs=1) as sb, \
             tc.tile_pool(name="xt_pool", bufs=3) as xt_pool, \
             tc.tile_pool(name="s_pool", bufs=2) as s_pool, \
             tc.tile_pool(name="nq_pool", bufs=1) as nq_pool, \
             tc.tile_pool(name="nr_pool", bufs=1) as nr_pool, \
             tc.tile_pool(name="u_pool", bufs=2) as u_pool, \
             tc.tile_pool(name="v_pool", bufs=1) as v_pool, \
             tc.tile_pool(name="o_pool", bufs=1) as o_pool, \
             tc.tile_pool(name="k_pool", bufs=2) as k_pool, \
             tc.tile_pool(name="h1_pool", bufs=2) as h1_pool, \
             tc.tile_pool(name="ht_pool", bufs=4) as ht_pool, \
             tc.tile_pool(name="f_pool", bufs=2) as f_pool, \
             tc.tile_pool(name="pp_s", bufs=2, space="PSUM") as pp_s, \
             tc.tile_pool(name="pp_tp", bufs=2, space="PSUM") as pp_tp, \
             tc.tile_pool(name="pp_h", bufs=2, space="PSUM") as pp_h:

            g = dict(sb=sb, s_pool=s_pool, nq_pool=nq_pool, nr_pool=nr_pool,
                     u_pool=u_pool, v_pool=v_pool, k_pool=k_pool,
                     h1_pool=h1_pool, ht_pool=ht_pool, f_pool=f_pool,
                     pp_s=pp_s, pp_tp=pp_tp, pp_h=pp_h)

            ident = sb.tile([P, P], F32, tag="ident")
            make_identity(nc, ident)
            g["ident"] = ident
            ones1 = sb.tile([1, P], F32, tag="ones1")
            nc.vector.memset(ones1[:], 1.0)
            g["ones1"] = ones1
            onescol = sb.tile([H, 1], F32, tag="onescol")
            nc.vector.memset(onescol[:], 1.0)
            g["onescol"] = onescol
            alpha64 = sb.tile([H, 1], F32, tag="alpha64")
            nc.vector.memset(alpha64[:], SLOPE)
            g["alpha64"] = alpha64
            alpha128 = sb.tile([P, 1], F32, tag="alpha128")
            nc.vector.memset(alpha128[:], SLOPE)
            g["alpha128"] = alpha128

            Ws = [_prep_weights(nc, g, li, 3 if li == 0 else H, wnames)
                  for li in range(3)]

            for rep in range(REPEAT):
                for c in range(NB):
                    # load cloud c's pos -> 2*X^T (rows 0..2)
                    x2t_a = xt_pool.tile([H, N], F32, tag="x2t")
                    xsb = u_pool.tile([P, NT * H], F32, tag="u")  # borrow
                    nc.sync.dma_start(
                        xsb[:, 0:NT * 3].rearrange("p (t d) -> p t d", d=3),
                        pos_d[c * N:(c + 1) * N].rearrange("(t p) d -> p t d", p=P))
                    for t in range(NT):
                        pt = pp_tp.tile([P, 512], F32, tag="tp")
                        nc.tensor.transpose(out=pt[0:3, 0:P],
                                            in_=xsb[:, t * 3:(t + 1) * 3],
                                            identity=ident[:])
                        nc.scalar.mul(out=x2t_a[0:3, t * P:(t + 1) * P],
                                      in_=pt[0:3, 0:P], mul=2.0)

                    x2t_b = xt_pool.tile([H, N], F32, tag="x2t")
                    _build_layer(nc, g, 0, 3, Ws[0], x2t_a, x2t_b, v_ds[0][c % 2])
                    x2t_c = xt_pool.tile([H, N], F32, tag="x2t")
                    _build_layer(nc, g, 1, H, Ws[1], x2t_b, x2t_c, v_ds[1][c % 2])
                    obuf = o_pool.tile([P, NT * H], F32, tag="obuf")
                    _build_layer(nc, g, 2, H, Ws[2], x2t_c, None, v_ds[2][c % 2],
                                 out_view=obuf)
                    nc.sync.dma_start(
                        out_d[c * N:(c + 1) * N].rearrange("(t p) f -> p t f", p=P),
                        obuf[:].rearrange("p (t f) -> p t f", f=H))
    nc.finalize()
    return nc


def pack_wblob(inputs):
    lay, tot = _blob_layout()
    blob = np.empty(tot, np.float32)
    for li in range(3):
        for nm, (off, shp) in lay[li].items():
            a = np.asarray(inputs[nm], dtype=np.float32).reshape(-1)
            blob[off:off + a.size] = a
    return blob


def make_in_maps(inputs):
    pos = np.ascontiguousarray(np.asarray(inputs["pos"], dtype=np.float32))
    blob = pack_wblob(inputs)
    pc = pos.reshape(NC, NB * N, 3)
    return [{"pos": pc[c], "wblob": blob} for c in range(NC)]


def _make_runner(nc):
    """Cached jitted NEFF executor: warm kernel() calls skip retracing.
    The weight blob is passed replicated (one host copy, not an NC-x concat)."""
    import jax
    from jax.sharding import Mesh, PartitionSpec
    from jax.experimental.shard_map import shard_map
    from concourse.bass2jax import (_bass_exec_p, install_neuronx_cc_hook,
                                    partition_id_tensor)
    install_neuronx_cc_hook()
    partition_name = nc.partition_id_tensor.name if nc.partition_id_tensor else None
    in_names, out_names, out_avals, zero_shapes = [], [], [], []
    for alloc in nc.m.functions[0].allocations:
        if not isinstance(alloc, mybir.MemoryLocationSet):
            continue
        name = alloc.memorylocations[0].name
        if alloc.kind == "ExternalInput":
            if name != partition_name:
                in_names.append(name)
        elif alloc.kind == "ExternalOutput":
            out_names.append(name)
            shape = tuple(alloc.tensor_shape)
            dtype = mybir.dt.np(alloc.dtype)
            out_avals.append(jax.core.ShapedArray(shape, dtype))
            zero_shapes.append((shape, dtype))
    n_params = len(in_names)
    n_outs = len(out_avals)
    in_names_all = list(in_names) + out_names
    if partition_name is not None:
        in_names_all.append(partition_name)

    def _body(*args):
        operands = list(args)
        if partition_name is not None:
            operands.append(partition_id_tensor())
        return tuple(_bass_exec_p.bind(
            *operands, out_avals=tuple(out_avals),
            in_names=tuple(in_names_all), out_names=tuple(out_names),
            lowering_input_output_aliases=(),
            sim_require_finite=True, sim_require_nnan=True, nc=nc))

    devices = jax.devices()[:NC]
    mesh = Mesh(np.asarray(devices), ("core",))
    rep_names = {"wblob"} & set(in_names)
    in_specs = tuple(
        PartitionSpec() if nm in rep_names else PartitionSpec("core")
        for nm in in_names
    ) + (PartitionSpec("core"),) * n_outs
    sharded = jax.jit(
        shard_map(_body, mesh=mesh, in_specs=in_specs,
                  out_specs=(PartitionSpec("core"),) * len(out_names),
                  check_rep=False),
        donate_argnums=tuple(range(n_params, n_params + n_outs)),
        keep_unused=True,
    )

    # Output buffers are pure scratch (the kernel fully overwrites out_d), so
    # they are chained across calls via donation: the first call ships zeros,
    # every later call re-donates the previous device-resident buffers. This
    # removes an 8 MB host->device upload (~80 ms of axon RPC) per warm call.
    state = {"outs": None}

    def submit(in_maps):
        """Upload fresh inputs, launch, return device output handles."""
        per_core = [[np.asarray(m[name]) for name in in_names] for m in in_maps]
        args_in = [
            per_core[0][i] if in_names[i] in rep_names else
            np.concatenate([per_core[c][i] for c in range(NC)], axis=0)
            for i in range(n_params)
        ]
        outs = state["outs"]
        if outs is None:
            outs = [np.zeros((NC * sh[0], *sh[1:]), dt) for sh, dt in zero_shapes]
        out_arrs = list(sharded(*args_in, *outs))
        state["outs"] = out_arrs
        jax.block_until_ready(out_arrs)
        return out_arrs

    def run(in_maps):
        out_arrs = submit(in_maps)
        return [
            {name: np.asarray(out_arrs[i]).reshape(NC, *out_avals[i].shape)[c]
             for i, name in enumerate(out_names)}
            for c in range(NC)
        ]

    run.submit = submit
    return run


def kernel(**inputs):
    if "nc" not in _CACHE:
        _CACHE["nc"] = build()
        _CACHE["run"] = _make_runner(_CACHE["nc"])
    in_maps = make_in_maps(inputs)
    results = _CACHE["run"](in_maps)
    out = np.concatenate([results[c]["out"] for c in range(NC)], axis=0)
    return out.reshape(B, N, H)


if __name__ == "__main__":
    rng = np.random.default_rng(0)
    fake = {"pos": rng.standard_normal((B, N, 3)).astype(np.float32)}
    for pfx in ("1", "2", "3"):
        d2 = 6 if pfx == "1" else 128
        fake[f"w{pfx}1"] = rng.standard_normal((d2, H)).astype(np.float32) * 0.2
        fake[f"w{pfx}2"] = rng.standard_normal((H, H)).astype(np.float32) * 0.12
        fake[f"w{pfx}3"] = rng.standard_normal((H, H)).astype(np.float32) * 0.12
        for j in ("1", "2", "3"):
            fake[f"b{pfx}{j}"] = np.zeros(H, np.float32)
    o = kernel(**fake)
    print("out", o.shape, o.dtype, float(np.abs(o).max()))


# revision 33
# speedup vs baseline: 1.3639x; 1.0035x over previous
"""DGCNN (3x DynamicEdgeConv, kNN=20) Trainium2 Bass kernel.

Self-contained: `kernel(**inputs) -> np.ndarray` takes the full inputs from
setup_inputs() (pos [8,4096,3] + 9 weight/bias pairs) and returns [8,4096,64].

Sharding: NC=2 NeuronCores, NB=4 whole point clouds per core, weights
replicated. Few cores is deliberate: per-call dispatch overhead on this axon
client scales ~5 ms per device in the sharded call, while one cloud's device
span is only ~4 ms, so 2 cores x 4 sequential clouds beats 8 x 1.

Per-core, per-cloud, per-layer pipeline (N=4096 points, D in {3,64}, H=64):
  phase 0: negsq2 = -2*||x_j||^2 row (ACT square + PE colsum);
           U = x@(W1a-W1b)+b1 (point-major, SBUF), V = x@W1b -> DRAM [4096,64]
  per row-tile t (128 points):
    S = 4 x_i.x_j - 2 sq_j  (PE, rank-1 ones x negsq2 accumulated into the
        same PSUM tile; row-monotone == -dist)
    top-20: 3 rounds of DVE max8 / max_index / match_replace -> 24 winners
    h1 = leaky(U_i + V_j): prefill U, 20x indirect-DMA gather with CCE-add
    h1 -> PE transposes -> h1T [64, 20*128] (edge k-major)
    h2T = Prelu(W2^T@h1T + b2), h3T = Prelu(W3^T@h2T + b3)   (PE + ACT)
    out tile = max over k (DVE strided reduce) -> next layer's 2*X^T
"""
import os
import numpy as np

import concourse.bass as bass
import concourse.bacc as bacc
import concourse.mybir as mybir
import concourse.tile as tile
from concourse.masks import make_identity

F32 = mybir.dt.float32
U32 = mybir.dt.uint32
AF = mybir.ActivationFunctionType
ALU = mybir.AluOpType

B = 8                  # total point clouds
NC = int(os.environ.get("NC", "2"))     # NeuronCores used
NB = B // NC           # clouds per core
NCORES = NC
N = 4096
P = 128
NT = N // P            # 32 row tiles
K = 20
H = 64
SLOPE = 0.2
NEG = -3.0e38
SKIP_GATHER = bool(int(os.environ.get("SKIP_GATHER", "0")))
SKIP_TOPK = bool(int(os.environ.get("SKIP_TOPK", "0")))
STAGE = int(os.environ.get("STAGE", "4"))  # truncate per-tile pipeline for attribution
SIM_ACT = bool(int(os.environ.get("SIM_ACT", "0")))  # Prelu->Copy for CoreSim
NSW = int(os.environ.get("NSW", "1"))      # SWDGE queues (extra queues cost ~10 ms/call each in per-call runtime setup -- keep 1)
REPEAT = int(os.environ.get("REPEAT", "1"))  # loop whole pipeline R times (timing)

_CACHE = {}


def _gather_q(nc, q, **kw):
    bi = nc.gpsimd.indirect_dma_start(**kw)
    if q:
        bi.ins.queue = f"qPoolDynamic{q}"
    return bi


def _prep_weights(nc, g, li, d_in, wn):
    """Load + derive one layer's weights into persistent SBUF tiles."""
    sb = g["sb"]
    w1_d, b1_d = wn[f"w{li+1}1"], wn[f"b{li+1}1"]
    w2_d, b2_d = wn[f"w{li+1}2"], wn[f"b{li+1}2"]
    w3_d, b3_d = wn[f"w{li+1}3"], wn[f"b{li+1}3"]
    wa = sb.tile([d_in, H], F32, tag=f"wa{li}")
    wb = sb.tile([d_in, H], F32, tag=f"wb{li}")
    nc.sync.dma_start(wa[:], w1_d[0:d_in, :])
    nc.sync.dma_start(wb[:], w1_d[d_in:2 * d_in, :])
    wd2 = sb.tile([d_in, H], F32, tag=f"wd2{li}")
    nc.vector.tensor_tensor(out=wd2[:], in0=wa[:], in1=wb[:], op=ALU.subtract)
    nc.vector.tensor_scalar_mul(wd2[:], wd2[:], 0.5)
    wb2 = sb.tile([d_in, H], F32, tag=f"wb2{li}")
    nc.vector.tensor_scalar_mul(wb2[:], wb[:], 0.5)
    w2 = sb.tile([H, H], F32, tag=f"w2{li}")
    nc.sync.dma_start(w2[:], w2_d[:])
    w3 = sb.tile([H, H], F32, tag=f"w3{li}")
    nc.sync.dma_start(w3[:], w3_d[:])
    b1r = sb.tile([1, H], F32, tag=f"b1r{li}")
    nc.sync.dma_start(b1r[:], b1_d[:].unsqueeze(0))
    b2c = sb.tile([H, 1], F32, tag=f"b2c{li}")
    nc.sync.dma_start(b2c[:], b2_d[:].unsqueeze(1))
    b3c = sb.tile([H, 1], F32, tag=f"b3c{li}")
    nc.sync.dma_start(b3c[:], b3_d[:].unsqueeze(1))
    return dict(wd2=wd2, wb2=wb2, w2=w2, w3=w3, b1r=b1r, b2c=b2c, b3c=b3c)


def _build_layer(nc, g, li, d_in, W, x2t, x2t_next, v_d, out_view=None):
    """Emit one EdgeConv layer for one cloud. x2t holds 2*X^T (rows 0..d_in-1).
    Writes 2*X_next^T into x2t_next, or the output tile rows into out_view."""
    pp_s, pp_tp, pp_h = g["pp_s"], g["pp_tp"], g["pp_h"]
    ident, ones1, alpha64, alpha128 = (
        g["ident"], g["ones1"], g["alpha64"], g["alpha128"])
    wd2, wb2, w2, w3 = W["wd2"], W["wb2"], W["w2"], W["w3"]
    b1r, b2c, b3c = W["b1r"], W["b2c"], W["b3c"]

    # ---- phase 0: x2r = S-matmul rhs: rows 0..d_in-1 = 2X^T, row d_in =
    # -2*sq (colsum(X2T^2) = 4 sq, scaled by -0.5). With the ones row kept at
    # x2t[d_in] for the lhsT, one matmul of contraction d_in+1 yields
    # 4 x_i.x_j - 2 sq_j directly -- no per-tile correction op at all, and
    # fp32 matmul cost is the 512-col stream length regardless of contraction.
    xsq = g["s_pool"].tile([P, N], F32, tag="s")   # borrow an S buffer
    nc.scalar.activation(out=xsq[0:d_in, :], in_=x2t[0:d_in, :], func=AF.Square)
    x2r = g["xr_pool"].tile([H + 1, N], F32, tag="x2r")
    nc.scalar.copy(out=x2r[0:d_in, :], in_=x2t[0:d_in, :])
    # ACT writes need a partition base of 0/32/64/96: layer 1's row 3 is
    # staged at base 64 (xsq row 64 is free) and moved by one SBUF DMA
    nq_view = x2r[d_in:d_in + 1, :] if d_in + 1 > 64 else xsq[64:65, :]
    for c in range(N // 512):
        ps = pp_tp.tile([P, 512], F32, tag="tp")
        nc.tensor.matmul(out=ps[0:1, :], lhsT=g["onescol"][0:d_in, :],
                         rhs=xsq[0:d_in, c * 512:(c + 1) * 512],
                         start=True, stop=True)
        nc.scalar.activation(out=nq_view[:, c * 512:(c + 1) * 512],
                             in_=ps[0:1, :], func=AF.Copy, scale=-0.5)
    if d_in + 1 <= 64:
        nc.sync.dma_start(x2r[d_in:d_in + 1, :], xsq[64:65, :])

    # ---- phase 0: U (point-major SBUF) and V (point-major DRAM) ----
    # 4 row tiles per PSUM tile; shared lhsT loads, then 4 bias matmuls with
    # a single ones-vector load; one wide ACT copy per quad
    u_sb = g["u_pool"].tile([P, NT * H], F32, tag="u")
    vbuf = g["v_pool"].tile([P, NT * H], F32, tag="vbuf")
    for t in range(NT):
        lhs = x2t[0:d_in, t * P:(t + 1) * P]
        pu = pp_tp.tile([P, 512], F32, tag="tp")
        nc.tensor.matmul(out=pu[:, 0:H], lhsT=lhs, rhs=wd2[:], start=True, stop=False)
        nc.tensor.matmul(out=pu[:, 0:H], lhsT=ones1[:, 0:P], rhs=b1r[:],
                         start=False, stop=True)
        nc.tensor.matmul(out=pu[:, H:2 * H], lhsT=lhs, rhs=wb2[:], start=True, stop=True)
        nc.scalar.copy(out=u_sb[:, t * H:(t + 1) * H], in_=pu[:, 0:H])
        nc.scalar.copy(out=vbuf[:, t * H:(t + 1) * H], in_=pu[:, H:2 * H])
    nc.sync.dma_start(v_d[:].rearrange("(t p) f -> p t f", p=P),
                      vbuf[:].rearrange("p (t f) -> p t f", f=H))

    if STAGE < 4:
        if out_view is None:
            nc.vector.memset(x2t_next[0:H, :], 0.5)
        else:
            nc.vector.memset(out_view[:], 0.5)
    if STAGE < 1:
        return

    # ---- per row-tile, software-pipelined 3 deep so no engine's in-order
    # stream stalls: S for tile t+1 is produced while DVE runs topk(t), and
    # the gather-dependent tail of tile t-1 (Prelu/transpose/MLP/reduce) runs
    # behind, after its gather DMAs have long landed. Without this the DVE
    # stream [topk(t), reduce(t)] idles ~35us/tile waiting on the MLP chain.
    st = {}  # per-tile live handles

    def emit_S(t):
        # distance tile S [128, N] (bigger = closer): one matmul per chunk,
        # one stationary load per tile, -2sq_j fused via the extra row
        s_sb = g["s_pool"].tile([P, N], F32, tag="s")
        for c in range(N // 1024):
            ps = pp_s.tile([P, 1024], F32, tag="dist")
            for hh in range(2):
                cc = c * 1024 + hh * 512
                nc.tensor.matmul(out=ps[:, hh * 512:(hh + 1) * 512],
                                 lhsT=x2t[0:d_in + 1, t * P:(t + 1) * P],
                                 rhs=x2r[0:d_in + 1, cc:cc + 512],
                                 start=True, stop=True)
            nc.scalar.copy(out=s_sb[:, c * 1024:(c + 1) * 1024], in_=ps[:])
        st[t] = {"s": s_sb}

    def emit_topk_gather(t):
        s_sb = st[t]["s"]
        # exact top-24 of each row with in-place knockout (8 wide DVE insts):
        # max8 -> indices -> replace-with-NEG, three rounds. match_replace
        # only zaps already-extracted values, so later max_index positions
        # in the modified S equal positions in the original.
        winners = g["k_pool"].tile([P, 24], F32, tag="win")
        idxt = g["k_pool"].tile([P, 24], U32, tag="idxt")
        if SKIP_TOPK:
            nc.vector.memset(winners[:], 0.0)
            nc.vector.memset(idxt[:], 0)
        else:
            for r in range(3):
                nc.vector.max(out=winners[:, r * 8:(r + 1) * 8], in_=s_sb[:])
                nc.vector.max_index(out=idxt[:, r * 8:(r + 1) * 8],
                                    in_max=winners[:, r * 8:(r + 1) * 8],
                                    in_values=s_sb[:])
                if r < 2:
                    nc.vector.match_replace(out=s_sb[:],
                                            in_to_replace=winners[:, r * 8:(r + 1) * 8],
                                            in_values=s_sb[:], imm_value=NEG)
        if STAGE < 2:
            return
        # edge features: h1 = leaky(U_i + V_j) -- prefill U, then CCE-add
        # gathers of V_j; the Prelu happens in the delayed tail
        h1 = g["h1_pool"].tile([P, K * H], F32, tag="h1")
        h1v = h1[:].rearrange("p (k f) -> p k f", k=K)
        u_bc = u_sb[:, t * H:(t + 1) * H].unsqueeze(1).to_broadcast([P, K, H])
        nc.scalar.copy(out=h1v, in_=u_bc)
        for k in (range(0) if SKIP_GATHER else range(K)):
            _gather_q(nc, k % NSW,
                      out=h1[:, k * H:(k + 1) * H], out_offset=None,
                      in_=v_d[:],
                      in_offset=bass.IndirectOffsetOnAxis(ap=idxt[:, k:k + 1], axis=0),
                      compute_op=ALU.add)
        st[t]["h1"] = h1

    def emit_tail(t):
        if STAGE < 2:
            st.pop(t, None)
            return
        h1 = st[t]["h1"]
        if SIM_ACT:
            nc.scalar.activation(out=h1[:], in_=h1[:], func=AF.Copy)
        else:
            nc.scalar.activation(out=h1[:], in_=h1[:], func=AF.Prelu, alpha=alpha128[:])
        if STAGE < 3:
            st.pop(t, None)
            return
        # transpose to edge-major h1T [64, k*128+i]
        h1t = g["ht_pool"].tile([H, K * P], F32, tag="ht")
        for kc in range(5):
            pt = pp_tp.tile([P, 512], F32, tag="tp")
            for j in range(4):
                k = kc * 4 + j
                nc.tensor.transpose(out=pt[0:H, j * P:(j + 1) * P],
                                    in_=h1[:, k * H:(k + 1) * H], identity=ident[:])
            nc.scalar.copy(out=h1t[:, kc * 512:(kc + 1) * 512], in_=pt[0:H, :])
        if STAGE < 4:
            st.pop(t, None)
            return
        # MLP layers 2, 3 (feature-major, edges on the free axis)
        h2t = g["ht_pool"].tile([H, K * P], F32, tag="ht")
        for e in range(5):
            ph = pp_h.tile([H, 512], F32, tag="h")
            nc.tensor.matmul(out=ph[:], lhsT=w2[:],
                             rhs=h1t[:, e * 512:(e + 1) * 512], start=True, stop=True)
            nc.scalar.activation(out=h2t[:, e * 512:(e + 1) * 512], in_=ph[:],
                                 func=AF.Copy if SIM_ACT else AF.Prelu,
                                 bias=0.0 if SIM_ACT else b2c[:],
                                 alpha=0.0 if SIM_ACT else alpha64[:])
        h3t = g["ht_pool"].tile([H, K * P], F32, tag="ht")
        for e in range(5):
            ph = pp_h.tile([H, 512], F32, tag="h")
            nc.tensor.matmul(out=ph[:], lhsT=w3[:],
                             rhs=h2t[:, e * 512:(e + 1) * 512], start=True, stop=True)
            nc.scalar.activation(out=h3t[:, e * 512:(e + 1) * 512], in_=ph[:],
                                 func=AF.Copy if SIM_ACT else AF.Prelu,
                                 bias=0.0 if SIM_ACT else b3c[:],
                                 alpha=0.0 if SIM_ACT else alpha64[:])
        # aggregate: max over k (innermost stride-128 axis)
        ftile = g["f_pool"].tile([H, P], F32, tag="f")
        nc.vector.tensor_reduce(out=ftile[:],
                                in_=h3t[:].rearrange("h (k i) -> h i k", k=K),
                                axis=mybir.AxisListType.X, op=ALU.max)
        if out_view is None:
            nc.scalar.mul(out=x2t_next[0:H, t * P:(t + 1) * P], in_=ftile[:], mul=2.0)
        else:
            po = pp_tp.tile([P, 512], F32, tag="tp")
            nc.tensor.transpose(out=po[:, 0:H], in_=ftile[:], identity=ident[0:H, 0:H])
            nc.scalar.copy(out=out_view[:, t * H:(t + 1) * H], in_=po[:, 0:H])
        st.pop(t, None)

    emit_S(0)
    for t in range(NT):
        if t + 1 < NT:
            emit_S(t + 1)
        emit_topk_gather(t)
        if t > 0:
            emit_tail(t - 1)
    emit_tail(NT - 1)


# weight-blob layout: per layer w1, w2, w3, b1, b2, b3 (fp32 elements).
# Packing all 18 weight tensors into ONE device input matters: each extra
# input tensor costs ~1.5 ms of per-call host/axon binding overhead.
def _blob_layout():
    off, lay = 0, []
    for li in range(3):
        d2 = 6 if li == 0 else 128
        ent = {}
        for nm, shp in ((f"w{li+1}1", (d2, H)), (f"w{li+1}2", (H, H)),
                        (f"w{li+1}3", (H, H)), (f"b{li+1}1", (H,)),
                        (f"b{li+1}2", (H,)), (f"b{li+1}3", (H,))):
            n = int(np.prod(shp))
            ent[nm] = (off, shp)
            off += n
        lay.append(ent)
    return lay, off


def build():
    nc = bacc.Bacc("TRN2", target_bir_lowering=False, debug=False,
                   num_swdge_queues=NSW)
    pos_d = nc.dram_tensor("pos", [NB * N, 3], F32, kind="ExternalInput")
    lay, tot = _blob_layout()
    wblob_d = nc.dram_tensor("wblob", [tot], F32, kind="ExternalInput")
    wnames = {}
    for li in range(3):
        for nm, (off, shp) in lay[li].items():
            n = int(np.prod(shp))
            v = wblob_d[off:off + n]
            if len(shp) == 2:
                v = v.rearrange("(r c) -> r c", c=shp[1])
            wnames[nm] = v
    out_d = nc.dram_tensor("out", [NB * N, H], F32, kind="ExternalOutput")
    # per-(layer, cloud-parity) V tables so adjacent clouds never share one
    v_ds = [[nc.dram_tensor(f"vtab{li}_{p}", [N, H], F32) for p in range(2)]
            for li in range(3)]

    with tile.TileContext(nc) as tc:
        with tc.tile_pool(name="sb", bufs=1) as sb, \
             tc.tile_pool(name="xt_pool", bufs=3) as xt_pool, \
             tc.tile_pool(name="s_pool", bufs=2) as s_pool, \
             tc.tile_pool(name="xr_pool", bufs=1) as xr_pool, \
             tc.tile_pool(name="u_pool", bufs=2) as u_pool, \
             tc.tile_pool(name="v_pool", bufs=1) as v_pool, \
             tc.tile_pool(name="o_pool", bufs=1) as o_pool, \
             tc.tile_pool(name="k_pool", bufs=2) as k_pool, \
             tc.tile_pool(name="h1_pool", bufs=2) as h1_pool, \
             tc.tile_pool(name="ht_pool", bufs=4) as ht_pool, \
             tc.tile_pool(name="f_pool", bufs=2) as f_pool, \
             tc.tile_pool(name="pp_s", bufs=2, space="PSUM") as pp_s, \
             tc.tile_pool(name="pp_tp", bufs=2, space="PSUM") as pp_tp, \
             tc.tile_pool(name="pp_h", bufs=2, space="PSUM") as pp_h:

            g = dict(sb=sb, s_pool=s_pool, xr_pool=xr_pool,
                     u_pool=u_pool, v_pool=v_pool, k_pool=k_pool,
                     h1_pool=h1_pool, ht_pool=ht_pool, f_pool=f_pool,
                     pp_s=pp_s, pp_tp=pp_tp, pp_h=pp_h)

            ident = sb.tile([P, P], F32, tag="ident")
            make_identity(nc, ident)
            g["ident"] = ident
            ones1 = sb.tile([1, P], F32, tag="ones1")
            nc.vector.memset(ones1[:], 1.0)
            g["ones1"] = ones1
            onescol = sb.tile([H, 1], F32, tag="onescol")
            nc.vector.memset(onescol[:], 1.0)
            g["onescol"] = onescol
            alpha64 = sb.tile([H, 1], F32, tag="alpha64")
            nc.vector.memset(alpha64[:], SLOPE)
            g["alpha64"] = alpha64
            alpha128 = sb.tile([P, 1], F32, tag="alpha128")
            nc.vector.memset(alpha128[:], SLOPE)
            g["alpha128"] = alpha128

            Ws = [_prep_weights(nc, g, li, 3 if li == 0 else H, wnames)
                  for li in range(3)]

            for rep in range(REPEAT):
                for c in range(NB):
                    # load cloud c's pos -> 2*X^T (rows 0..2); row 3 =
                    # ones (the S-matmul lhsT extra row; layers 2/3 use 64)
                    x2t_a = xt_pool.tile([H + 1, N], F32, tag="x2t")
                    # rows 0..2 are overwritten by the pos transposes below;
                    # row 3 stays 1 (gpsimd needs a partition base of 0)
                    nc.gpsimd.memset(x2t_a[0:4, :], 1.0)
                    xsb = u_pool.tile([P, NT * H], F32, tag="u")  # borrow
                    nc.sync.dma_start(
                        xsb[:, 0:NT * 3].rearrange("p (t d) -> p t d", d=3),
                        pos_d[c * N:(c + 1) * N].rearrange("(t p) d -> p t d", p=P))
                    for t in range(NT):
                        pt = pp_tp.tile([P, 512], F32, tag="tp")
                        nc.tensor.transpose(out=pt[0:3, 0:P],
                                            in_=xsb[:, t * 3:(t + 1) * 3],
                                            identity=ident[:])
                        nc.scalar.mul(out=x2t_a[0:3, t * P:(t + 1) * P],
                                      in_=pt[0:3, 0:P], mul=2.0)

                    x2t_b = xt_pool.tile([H + 1, N], F32, tag="x2t")
                    nc.gpsimd.memset(x2t_b[H:H + 1, :], 1.0)
                    _build_layer(nc, g, 0, 3, Ws[0], x2t_a, x2t_b, v_ds[0][c % 2])
                    x2t_c = xt_pool.tile([H + 1, N], F32, tag="x2t")
                    nc.gpsimd.memset(x2t_c[H:H + 1, :], 1.0)
                    _build_layer(nc, g, 1, H, Ws[1], x2t_b, x2t_c, v_ds[1][c % 2])
                    obuf = o_pool.tile([P, NT * H], F32, tag="obuf")
                    _build_layer(nc, g, 2, H, Ws[2], x2t_c, None, v_ds[2][c % 2],
                                 out_view=obuf)
                    nc.sync.dma_start(
                        out_d[c * N:(c + 1) * N].rearrange("(t p) f -> p t f", p=P),
                        obuf[:].rearrange("p (t f) -> p t f", f=H))
    nc.finalize()
    return nc


def pack_wblob(inputs):
    lay, tot = _blob_layout()
    blob = np.empty(tot, np.float32)
    for li in range(3):
        for nm, (off, shp) in lay[li].items():
            a = np.asarray(inputs[nm], dtype=np.float32).reshape(-1)
            blob[off:off + a.size] = a
    return blob


def make_in_maps(inputs):
    pos = np.ascontiguousarray(np.asarray(inputs["pos"], dtype=np.float32))
    blob = pack_wblob(inputs)
    pc = pos.reshape(NC, NB * N, 3)
    return [{"pos": pc[c], "wblob": blob} for c in range(NC)]


def _make_runner(nc):
    """Cached jitted NEFF executor: warm kernel() calls skip retracing.
    The weight blob is passed replicated (one host copy, not an NC-x concat)."""
    import jax
    from jax.sharding import Mesh, PartitionSpec
    from jax.experimental.shard_map import shard_map
    from concourse.bass2jax import (_bass_exec_p, install_neuronx_cc_hook,
                                    partition_id_tensor)
    install_neuronx_cc_hook()
    partition_name = nc.partition_id_tensor.name if nc.partition_id_tensor else None
    in_names, out_names, out_avals, zero_shapes = [], [], [], []
    for alloc in nc.m.functions[0].allocations:
        if not isinstance(alloc, mybir.MemoryLocationSet):
            continue
        name = alloc.memorylocations[0].name
        if alloc.kind == "ExternalInput":
            if name != partition_name:
                in_names.append(name)
        elif alloc.kind == "ExternalOutput":
            out_names.append(name)
            shape = tuple(alloc.tensor_shape)
            dtype = mybir.dt.np(alloc.dtype)
            out_avals.append(jax.core.ShapedArray(shape, dtype))
            zero_shapes.append((shape, dtype))
    n_params = len(in_names)
    n_outs = len(out_avals)
    in_names_all = list(in_names) + out_names
    if partition_name is not None:
        in_names_all.append(partition_name)

    def _body(*args):
        operands = list(args)
        if partition_name is not None:
            operands.append(partition_id_tensor())
        return tuple(_bass_exec_p.bind(
            *operands, out_avals=tuple(out_avals),
            in_names=tuple(in_names_all), out_names=tuple(out_names),
            lowering_input_output_aliases=(),
            sim_require_finite=True, sim_require_nnan=True, nc=nc))

    devices = jax.devices()[:NC]
    mesh = Mesh(np.asarray(devices), ("core",))
    rep_names = {"wblob"} & set(in_names)
    in_specs = tuple(
        PartitionSpec() if nm in rep_names else PartitionSpec("core")
        for nm in in_names
    ) + (PartitionSpec("core"),) * n_outs
    sharded = jax.jit(
        shard_map(_body, mesh=mesh, in_specs=in_specs,
                  out_specs=(PartitionSpec("core"),) * len(out_names),
                  check_rep=False),
        donate_argnums=tuple(range(n_params, n_params + n_outs)),
        keep_unused=True,
    )

    # Output buffers are pure scratch (the kernel fully overwrites out_d), so
    # they are chained across calls via donation: the first call ships zeros,
    # every later call re-donates the previous device-resident buffers. This
    # removes an 8 MB host->device upload (~80 ms of axon RPC) per warm call.
    state = {"outs": None}

    def submit(in_maps):
        """Upload fresh inputs, launch, return device output handles."""
        per_core = [[np.asarray(m[name]) for name in in_names] for m in in_maps]
        args_in = [
            per_core[0][i] if in_names[i] in rep_names else
            np.concatenate([per_core[c][i] for c in range(NC)], axis=0)
            for i in range(n_params)
        ]
        outs = state["outs"]
        if outs is None:
            outs = [np.zeros((NC * sh[0], *sh[1:]), dt) for sh, dt in zero_shapes]
        out_arrs = list(sharded(*args_in, *outs))
        state["outs"] = out_arrs
        jax.block_until_ready(out_arrs)
        return out_arrs

    def run(in_maps):
        out_arrs = submit(in_maps)
        return [
            {name: np.asarray(out_arrs[i]).reshape(NC, *out_avals[i].shape)[c]
             for i, name in enumerate(out_names)}
            for c in range(NC)
        ]

    run.submit = submit
    return run


def kernel(**inputs):
    if "nc" not in _CACHE:
        _CACHE["nc"] = build()
        _CACHE["run"] = _make_runner(_CACHE["nc"])
    in_maps = make_in_maps(inputs)
    results = _CACHE["run"](in_maps)
    out = np.concatenate([results[c]["out"] for c in range(NC)], axis=0)
    return out.reshape(B, N, H)


if __name__ == "__main__":
    rng = np.random.default_rng(0)
    fake = {"pos": rng.standard_normal((B, N, 3)).astype(np.float32)}
    for pfx in ("1", "2", "3"):
        d2 = 6 if pfx == "1" else 128
        fake[f"w{pfx}1"] = rng.standard_normal((d2, H)).astype(np.float32) * 0.2
        fake[f"w{pfx}2"] = rng.standard_normal((H, H)).astype(np.float32) * 0.12
        fake[f"w{pfx}3"] = rng.standard_normal((H, H)).astype(np.float32) * 0.12
        for j in ("1", "2", "3"):
            fake[f"b{pfx}{j}"] = np.zeros(H, np.float32)
    o = kernel(**fake)
    print("out", o.shape, o.dtype, float(np.abs(o).max()))


# revision 35
# speedup vs baseline: 1.4574x; 1.0685x over previous
"""DGCNN (3x DynamicEdgeConv, kNN=20) Trainium2 Bass kernel.

Self-contained: `kernel(**inputs) -> np.ndarray` takes the full inputs from
setup_inputs() (pos [8,4096,3] + 9 weight/bias pairs) and returns [8,4096,64].

Sharding: NC=2 NeuronCores, NB=4 whole point clouds per core, weights
replicated. Few cores is deliberate: per-call dispatch overhead on this axon
client scales ~5 ms per device in the sharded call, while one cloud's device
span is only ~4 ms, so 2 cores x 4 sequential clouds beats 8 x 1.

Per-core, per-cloud, per-layer pipeline (N=4096 points, D in {3,64}, H=64):
  phase 0: negsq2 = -2*||x_j||^2 row (ACT square + PE colsum);
           U = x@(W1a-W1b)+b1 (point-major, SBUF), V = x@W1b -> DRAM [4096,64]
  per row-tile t (128 points):
    S = 4 x_i.x_j - 2 sq_j  (PE, rank-1 ones x negsq2 accumulated into the
        same PSUM tile; row-monotone == -dist)
    top-20: 3 rounds of DVE max8 / max_index / match_replace -> 24 winners
    h1 = leaky(U_i + V_j): prefill U, 20x indirect-DMA gather with CCE-add
    h1 -> PE transposes -> h1T [64, 20*128] (edge k-major)
    h2T = Prelu(W2^T@h1T + b2), h3T = Prelu(W3^T@h2T + b3)   (PE + ACT)
    out tile = max over k (DVE strided reduce) -> next layer's 2*X^T
"""
import os
import numpy as np

import concourse.bass as bass
import concourse.bacc as bacc
import concourse.mybir as mybir
import concourse.tile as tile
from concourse.masks import make_identity

F32 = mybir.dt.float32
U32 = mybir.dt.uint32
AF = mybir.ActivationFunctionType
ALU = mybir.AluOpType

B = 8                  # total point clouds
NC = int(os.environ.get("NC", "2"))     # NeuronCores used
NB = B // NC           # clouds per core
NCORES = NC
N = 4096
P = 128
NT = N // P            # 32 row tiles
K = 20
H = 64
SLOPE = 0.2
NEG = -3.0e38
SKIP_GATHER = bool(int(os.environ.get("SKIP_GATHER", "0")))
SKIP_TOPK = bool(int(os.environ.get("SKIP_TOPK", "0")))
STAGE = int(os.environ.get("STAGE", "4"))  # truncate per-tile pipeline for attribution
SIM_ACT = bool(int(os.environ.get("SIM_ACT", "0")))  # Prelu->Copy for CoreSim
NSW = int(os.environ.get("NSW", "1"))      # SWDGE queues (extra queues cost ~10 ms/call each in per-call runtime setup -- keep 1)
REPEAT = int(os.environ.get("REPEAT", "1"))  # loop whole pipeline R times (timing)

_CACHE = {}


def _gather_q(nc, q, **kw):
    bi = nc.gpsimd.indirect_dma_start(**kw)
    if q:
        bi.ins.queue = f"qPoolDynamic{q}"
    return bi


def _prep_weights(nc, g, li, d_in, wn):
    """Load + derive one layer's weights into persistent SBUF tiles."""
    sb = g["sb"]
    w1_d, b1_d = wn[f"w{li+1}1"], wn[f"b{li+1}1"]
    w2_d, b2_d = wn[f"w{li+1}2"], wn[f"b{li+1}2"]
    w3_d, b3_d = wn[f"w{li+1}3"], wn[f"b{li+1}3"]
    wa = sb.tile([d_in, H], F32, tag=f"wa{li}")
    wb = sb.tile([d_in, H], F32, tag=f"wb{li}")
    nc.sync.dma_start(wa[:], w1_d[0:d_in, :])
    nc.sync.dma_start(wb[:], w1_d[d_in:2 * d_in, :])
    wd2 = sb.tile([d_in, H], F32, tag=f"wd2{li}")
    nc.vector.tensor_tensor(out=wd2[:], in0=wa[:], in1=wb[:], op=ALU.subtract)
    nc.vector.tensor_scalar_mul(wd2[:], wd2[:], 0.5)
    wb2 = sb.tile([d_in, H], F32, tag=f"wb2{li}")
    nc.vector.tensor_scalar_mul(wb2[:], wb[:], 0.5)
    w2 = sb.tile([H, H], F32, tag=f"w2{li}")
    nc.sync.dma_start(w2[:], w2_d[:])
    w3 = sb.tile([H, H], F32, tag=f"w3{li}")
    nc.sync.dma_start(w3[:], w3_d[:])
    b1r = sb.tile([1, H], F32, tag=f"b1r{li}")
    nc.sync.dma_start(b1r[:], b1_d[:].unsqueeze(0))
    b2c = sb.tile([H, 1], F32, tag=f"b2c{li}")
    nc.sync.dma_start(b2c[:], b2_d[:].unsqueeze(1))
    b3c = sb.tile([H, 1], F32, tag=f"b3c{li}")
    nc.sync.dma_start(b3c[:], b3_d[:].unsqueeze(1))
    return dict(wd2=wd2, wb2=wb2, w2=w2, w3=w3, b1r=b1r, b2c=b2c, b3c=b3c)


def _build_layer(nc, g, li, d_in, W, x2t, x2t_next, v_d, out_view=None):
    """Emit one EdgeConv layer for one cloud. x2t holds 2*X^T (rows 0..d_in-1).
    Writes 2*X_next^T into x2t_next, or the output tile rows into out_view."""
    pp_s, pp_tp, pp_h = g["pp_s"], g["pp_tp"], g["pp_h"]
    ident, ones1, alpha64, alpha128 = (
        g["ident"], g["ones1"], g["alpha64"], g["alpha128"])
    wd2, wb2, w2, w3 = W["wd2"], W["wb2"], W["w2"], W["w3"]
    b1r, b2c, b3c = W["b1r"], W["b2c"], W["b3c"]

    # ---- phase 0: x2r = S-matmul rhs: rows 0..d_in-1 = 2X^T, row d_in =
    # -2*sq (colsum(X2T^2) = 4 sq, scaled by -0.5). With the ones row kept at
    # x2t[d_in] for the lhsT, one matmul of contraction d_in+1 yields
    # 4 x_i.x_j - 2 sq_j directly -- no per-tile correction op at all, and
    # fp32 matmul cost is the 512-col stream length regardless of contraction.
    xsq = g["s_pool"].tile([P, N], F32, tag="s")   # borrow an S buffer
    nc.scalar.activation(out=xsq[0:d_in, :], in_=x2t[0:d_in, :], func=AF.Square)
    x2r = g["xr_pool"].tile([H + 1, N], F32, tag="x2r")
    nc.scalar.copy(out=x2r[0:d_in, :], in_=x2t[0:d_in, :])
    # ACT writes need a partition base of 0/32/64/96: layer 1's row 3 is
    # staged at base 64 (xsq row 64 is free) and moved by one SBUF DMA
    nq_view = x2r[d_in:d_in + 1, :] if d_in + 1 > 64 else xsq[64:65, :]
    for c in range(N // 512):
        ps = pp_tp.tile([P, 512], F32, tag="tp")
        nc.tensor.matmul(out=ps[0:1, :], lhsT=g["onescol"][0:d_in, :],
                         rhs=xsq[0:d_in, c * 512:(c + 1) * 512],
                         start=True, stop=True)
        nc.scalar.activation(out=nq_view[:, c * 512:(c + 1) * 512],
                             in_=ps[0:1, :], func=AF.Copy, scale=-0.5)
    if d_in + 1 <= 64:
        nc.sync.dma_start(x2r[d_in:d_in + 1, :], xsq[64:65, :])

    # ---- phase 0: U (point-major SBUF) and V (point-major DRAM) ----
    # 4 row tiles per PSUM tile; shared lhsT loads, then 4 bias matmuls with
    # a single ones-vector load; one wide ACT copy per quad
    u_sb = g["u_pool"].tile([P, NT * H], F32, tag="u")
    vbuf = g["v_pool"].tile([P, NT * H], F32, tag="vbuf")
    for t in range(NT):
        lhs = x2t[0:d_in, t * P:(t + 1) * P]
        pu = pp_tp.tile([P, 512], F32, tag="tp")
        nc.tensor.matmul(out=pu[:, 0:H], lhsT=lhs, rhs=wd2[:], start=True, stop=False)
        nc.tensor.matmul(out=pu[:, 0:H], lhsT=ones1[:, 0:P], rhs=b1r[:],
                         start=False, stop=True)
        nc.tensor.matmul(out=pu[:, H:2 * H], lhsT=lhs, rhs=wb2[:], start=True, stop=True)
        nc.scalar.copy(out=u_sb[:, t * H:(t + 1) * H], in_=pu[:, 0:H])
        nc.scalar.copy(out=vbuf[:, t * H:(t + 1) * H], in_=pu[:, H:2 * H])
    nc.sync.dma_start(v_d[:].rearrange("(t p) f -> p t f", p=P),
                      vbuf[:].rearrange("p (t f) -> p t f", f=H))

    if STAGE < 4:
        if out_view is None:
            nc.vector.memset(x2t_next[0:H, :], 0.5)
        else:
            nc.vector.memset(out_view[:], 0.5)
    if STAGE < 1:
        return

    # ---- per row-tile, software-pipelined 3 deep so no engine's in-order
    # stream stalls: S for tile t+1 is produced while DVE runs topk(t), and
    # the gather-dependent tail of tile t-1 (Prelu/transpose/MLP/reduce) runs
    # behind, after its gather DMAs have long landed. Without this the DVE
    # stream [topk(t), reduce(t)] idles ~35us/tile waiting on the MLP chain.
    st = {}  # per-tile live handles

    def emit_S(t):
        # distance tile S [128, N] (bigger = closer): one matmul per chunk,
        # one stationary load per tile, -2sq_j fused via the extra row
        s_sb = g["s_pool"].tile([P, N], F32, tag="s")
        for c in range(N // 1024):
            ps = pp_s.tile([P, 1024], F32, tag="dist")
            for hh in range(2):
                cc = c * 1024 + hh * 512
                nc.tensor.matmul(out=ps[:, hh * 512:(hh + 1) * 512],
                                 lhsT=x2t[0:d_in + 1, t * P:(t + 1) * P],
                                 rhs=x2r[0:d_in + 1, cc:cc + 512],
                                 start=True, stop=True)
            nc.scalar.copy(out=s_sb[:, c * 1024:(c + 1) * 1024], in_=ps[:])
        st[t] = {"s": s_sb}

    def emit_topk_gather(t):
        s_sb = st[t]["s"]
        # exact top-24 of each row with in-place knockout (8 wide DVE insts):
        # max8 -> indices -> replace-with-NEG, three rounds. match_replace
        # only zaps already-extracted values, so later max_index positions
        # in the modified S equal positions in the original.
        winners = g["k_pool"].tile([P, 24], F32, tag="win")
        idxt = g["k_pool"].tile([P, 24], U32, tag="idxt")
        if SKIP_TOPK:
            nc.vector.memset(winners[:], 0.0)
            nc.vector.memset(idxt[:], 0)
        else:
            for r in range(3):
                nc.vector.max(out=winners[:, r * 8:(r + 1) * 8], in_=s_sb[:])
                nc.vector.max_index(out=idxt[:, r * 8:(r + 1) * 8],
                                    in_max=winners[:, r * 8:(r + 1) * 8],
                                    in_values=s_sb[:])
                if r < 2:
                    nc.vector.match_replace(out=s_sb[:],
                                            in_to_replace=winners[:, r * 8:(r + 1) * 8],
                                            in_values=s_sb[:], imm_value=NEG)
        if STAGE < 2:
            return
        # edge features: h1 = leaky(U_i + V_j) -- prefill U, then CCE-add
        # gathers of V_j; the Prelu happens in the delayed tail
        h1 = g["h1_pool"].tile([P, K * H], F32, tag="h1")
        h1v = h1[:].rearrange("p (k f) -> p k f", k=K)
        u_bc = u_sb[:, t * H:(t + 1) * H].unsqueeze(1).to_broadcast([P, K, H])
        nc.scalar.copy(out=h1v, in_=u_bc)
        for k in (range(0) if SKIP_GATHER else range(K)):
            _gather_q(nc, k % NSW,
                      out=h1[:, k * H:(k + 1) * H], out_offset=None,
                      in_=v_d[:],
                      in_offset=bass.IndirectOffsetOnAxis(ap=idxt[:, k:k + 1], axis=0),
                      compute_op=ALU.add)
        st[t]["h1"] = h1

    def emit_tail(t):
        if STAGE < 2:
            st.pop(t, None)
            return
        h1 = st[t]["h1"]
        if SIM_ACT:
            nc.scalar.activation(out=h1[:], in_=h1[:], func=AF.Copy)
        else:
            nc.scalar.activation(out=h1[:], in_=h1[:], func=AF.Prelu, alpha=alpha128[:])
        if STAGE < 3:
            st.pop(t, None)
            return
        # transpose to edge-major h1T [64, k*128+i]
        h1t = g["ht_pool"].tile([H, K * P], F32, tag="ht")
        for kc in range(5):
            pt = pp_tp.tile([P, 512], F32, tag="tp")
            for j in range(4):
                k = kc * 4 + j
                nc.tensor.transpose(out=pt[0:H, j * P:(j + 1) * P],
                                    in_=h1[:, k * H:(k + 1) * H], identity=ident[:])
            nc.scalar.copy(out=h1t[:, kc * 512:(kc + 1) * 512], in_=pt[0:H, :])
        if STAGE < 4:
            st.pop(t, None)
            return
        # MLP layers 2, 3 (feature-major, edges on the free axis)
        h2t = g["ht_pool"].tile([H, K * P], F32, tag="ht")
        for e in range(5):
            ph = pp_h.tile([H, 512], F32, tag="h")
            nc.tensor.matmul(out=ph[:], lhsT=w2[:],
                             rhs=h1t[:, e * 512:(e + 1) * 512], start=True, stop=True)
            nc.scalar.activation(out=h2t[:, e * 512:(e + 1) * 512], in_=ph[:],
                                 func=AF.Copy if SIM_ACT else AF.Prelu,
                                 bias=0.0 if SIM_ACT else b2c[:],
                                 alpha=0.0 if SIM_ACT else alpha64[:])
        h3t = g["ht_pool"].tile([H, K * P], F32, tag="ht")
        for e in range(5):
            ph = pp_h.tile([H, 512], F32, tag="h")
            nc.tensor.matmul(out=ph[:], lhsT=w3[:],
                             rhs=h2t[:, e * 512:(e + 1) * 512], start=True, stop=True)
            nc.scalar.activation(out=h3t[:, e * 512:(e + 1) * 512], in_=ph[:],
                                 func=AF.Copy if SIM_ACT else AF.Prelu,
                                 bias=0.0 if SIM_ACT else b3c[:],
                                 alpha=0.0 if SIM_ACT else alpha64[:])
        # aggregate: max over k (innermost stride-128 axis). Stays on DVE:
        # the Pool engine's TensorTensor rejects the max opcode at codegen
        # (add works), so a Pool-side pairwise-max tree is not available.
        ftile = g["f_pool"].tile([H, P], F32, tag="f")
        nc.vector.tensor_reduce(out=ftile[:],
                                in_=h3t[:].rearrange("h (k i) -> h i k", k=K),
                                axis=mybir.AxisListType.X, op=ALU.max)
        if out_view is None:
            nc.scalar.mul(out=x2t_next[0:H, t * P:(t + 1) * P], in_=ftile[:], mul=2.0)
        else:
            po = pp_tp.tile([P, 512], F32, tag="tp")
            nc.tensor.transpose(out=po[:, 0:H], in_=ftile[:], identity=ident[0:H, 0:H])
            nc.scalar.copy(out=out_view[:, t * H:(t + 1) * H], in_=po[:, 0:H])
        st.pop(t, None)

    emit_S(0)
    for t in range(NT):
        if t + 1 < NT:
            emit_S(t + 1)
        emit_topk_gather(t)
        if t > 0:
            emit_tail(t - 1)
    emit_tail(NT - 1)


# weight-blob layout: per layer w1, w2, w3, b1, b2, b3 (fp32 elements).
# Packing all 18 weight tensors into ONE device input matters: each extra
# input tensor costs ~1.5 ms of per-call host/axon binding overhead.
def _blob_layout():
    off, lay = 0, []
    for li in range(3):
        d2 = 6 if li == 0 else 128
        ent = {}
        for nm, shp in ((f"w{li+1}1", (d2, H)), (f"w{li+1}2", (H, H)),
                        (f"w{li+1}3", (H, H)), (f"b{li+1}1", (H,)),
                        (f"b{li+1}2", (H,)), (f"b{li+1}3", (H,))):
            n = int(np.prod(shp))
            ent[nm] = (off, shp)
            off += n
        lay.append(ent)
    return lay, off


def build():
    nc = bacc.Bacc("TRN2", target_bir_lowering=False, debug=False,
                   num_swdge_queues=NSW)
    pos_d = nc.dram_tensor("pos", [NB * N, 3], F32, kind="ExternalInput")
    lay, tot = _blob_layout()
    wblob_d = nc.dram_tensor("wblob", [tot], F32, kind="ExternalInput")
    wnames = {}
    for li in range(3):
        for nm, (off, shp) in lay[li].items():
            n = int(np.prod(shp))
            v = wblob_d[off:off + n]
            if len(shp) == 2:
                v = v.rearrange("(r c) -> r c", c=shp[1])
            wnames[nm] = v
    out_d = nc.dram_tensor("out", [NB * N, H], F32, kind="ExternalOutput")
    # per-(layer, cloud-parity) V tables so adjacent clouds never share one
    v_ds = [[nc.dram_tensor(f"vtab{li}_{p}", [N, H], F32) for p in range(2)]
            for li in range(3)]

    with tile.TileContext(nc) as tc:
        with tc.tile_pool(name="sb", bufs=1) as sb, \
             tc.tile_pool(name="xt_pool", bufs=3) as xt_pool, \
             tc.tile_pool(name="s_pool", bufs=2) as s_pool, \
             tc.tile_pool(name="xr_pool", bufs=1) as xr_pool, \
             tc.tile_pool(name="u_pool", bufs=2) as u_pool, \
             tc.tile_pool(name="v_pool", bufs=1) as v_pool, \
             tc.tile_pool(name="o_pool", bufs=1) as o_pool, \
             tc.tile_pool(name="k_pool", bufs=2) as k_pool, \
             tc.tile_pool(name="h1_pool", bufs=2) as h1_pool, \
             tc.tile_pool(name="ht_pool", bufs=4) as ht_pool, \
             tc.tile_pool(name="f_pool", bufs=2) as f_pool, \
             tc.tile_pool(name="pp_s", bufs=2, space="PSUM") as pp_s, \
             tc.tile_pool(name="pp_tp", bufs=2, space="PSUM") as pp_tp, \
             tc.tile_pool(name="pp_h", bufs=2, space="PSUM") as pp_h:

            g = dict(sb=sb, s_pool=s_pool, xr_pool=xr_pool,
                     u_pool=u_pool, v_pool=v_pool, k_pool=k_pool,
                     h1_pool=h1_pool, ht_pool=ht_pool, f_pool=f_pool,
                     pp_s=pp_s, pp_tp=pp_tp, pp_h=pp_h)

            ident = sb.tile([P, P], F32, tag="ident")
            make_identity(nc, ident)
            g["ident"] = ident
            ones1 = sb.tile([1, P], F32, tag="ones1")
            nc.vector.memset(ones1[:], 1.0)
            g["ones1"] = ones1
            onescol = sb.tile([H, 1], F32, tag="onescol")
            nc.vector.memset(onescol[:], 1.0)
            g["onescol"] = onescol
            alpha64 = sb.tile([H, 1], F32, tag="alpha64")
            nc.vector.memset(alpha64[:], SLOPE)
            g["alpha64"] = alpha64
            alpha128 = sb.tile([P, 1], F32, tag="alpha128")
            nc.vector.memset(alpha128[:], SLOPE)
            g["alpha128"] = alpha128

            Ws = [_prep_weights(nc, g, li, 3 if li == 0 else H, wnames)
                  for li in range(3)]

            for rep in range(REPEAT):
                for c in range(NB):
                    # load cloud c's pos -> 2*X^T (rows 0..2); row 3 =
                    # ones (the S-matmul lhsT extra row; layers 2/3 use 64)
                    x2t_a = xt_pool.tile([H + 1, N], F32, tag="x2t")
                    # rows 0..2 are overwritten by the pos transposes below;
                    # row 3 stays 1 (gpsimd needs a partition base of 0)
                    nc.gpsimd.memset(x2t_a[0:4, :], 1.0)
                    xsb = u_pool.tile([P, NT * H], F32, tag="u")  # borrow
                    nc.sync.dma_start(
                        xsb[:, 0:NT * 3].rearrange("p (t d) -> p t d", d=3),
                        pos_d[c * N:(c + 1) * N].rearrange("(t p) d -> p t d", p=P))
                    for t in range(NT):
                        pt = pp_tp.tile([P, 512], F32, tag="tp")
                        nc.tensor.transpose(out=pt[0:3, 0:P],
                                            in_=xsb[:, t * 3:(t + 1) * 3],
                                            identity=ident[:])
                        nc.scalar.mul(out=x2t_a[0:3, t * P:(t + 1) * P],
                                      in_=pt[0:3, 0:P], mul=2.0)

                    x2t_b = xt_pool.tile([H + 1, N], F32, tag="x2t")
                    nc.gpsimd.memset(x2t_b[H:H + 1, :], 1.0)
                    _build_layer(nc, g, 0, 3, Ws[0], x2t_a, x2t_b, v_ds[0][c % 2])
                    x2t_c = xt_pool.tile([H + 1, N], F32, tag="x2t")
                    nc.gpsimd.memset(x2t_c[H:H + 1, :], 1.0)
                    _build_layer(nc, g, 1, H, Ws[1], x2t_b, x2t_c, v_ds[1][c % 2])
                    obuf = o_pool.tile([P, NT * H], F32, tag="obuf")
                    _build_layer(nc, g, 2, H, Ws[2], x2t_c, None, v_ds[2][c % 2],
                                 out_view=obuf)
                    nc.sync.dma_start(
                        out_d[c * N:(c + 1) * N].rearrange("(t p) f -> p t f", p=P),
                        obuf[:].rearrange("p (t f) -> p t f", f=H))
    nc.finalize()
    return nc


def pack_wblob(inputs):
    lay, tot = _blob_layout()
    blob = np.empty(tot, np.float32)
    for li in range(3):
        for nm, (off, shp) in lay[li].items():
            a = np.asarray(inputs[nm], dtype=np.float32).reshape(-1)
            blob[off:off + a.size] = a
    return blob


def make_in_maps(inputs):
    pos = np.ascontiguousarray(np.asarray(inputs["pos"], dtype=np.float32))
    blob = pack_wblob(inputs)
    pc = pos.reshape(NC, NB * N, 3)
    return [{"pos": pc[c], "wblob": blob} for c in range(NC)]


def _make_runner(nc):
    """Cached jitted NEFF executor: warm kernel() calls skip retracing.
    The weight blob is passed replicated (one host copy, not an NC-x concat)."""
    import jax
    from jax.sharding import Mesh, PartitionSpec
    from jax.experimental.shard_map import shard_map
    from concourse.bass2jax import (_bass_exec_p, install_neuronx_cc_hook,
                                    partition_id_tensor)
    install_neuronx_cc_hook()
    partition_name = nc.partition_id_tensor.name if nc.partition_id_tensor else None
    in_names, out_names, out_avals, zero_shapes = [], [], [], []
    for alloc in nc.m.functions[0].allocations:
        if not isinstance(alloc, mybir.MemoryLocationSet):
            continue
        name = alloc.memorylocations[0].name
        if alloc.kind == "ExternalInput":
            if name != partition_name:
                in_names.append(name)
        elif alloc.kind == "ExternalOutput":
            out_names.append(name)
            shape = tuple(alloc.tensor_shape)
            dtype = mybir.dt.np(alloc.dtype)
            out_avals.append(jax.core.ShapedArray(shape, dtype))
            zero_shapes.append((shape, dtype))
    n_params = len(in_names)
    n_outs = len(out_avals)
    in_names_all = list(in_names) + out_names
    if partition_name is not None:
        in_names_all.append(partition_name)

    def _body(*args):
        operands = list(args)
        if partition_name is not None:
            operands.append(partition_id_tensor())
        return tuple(_bass_exec_p.bind(
            *operands, out_avals=tuple(out_avals),
            in_names=tuple(in_names_all), out_names=tuple(out_names),
            lowering_input_output_aliases=(),
            sim_require_finite=True, sim_require_nnan=True, nc=nc))

    devices = jax.devices()[:NC]
    mesh = Mesh(np.asarray(devices), ("core",))
    rep_names = {"wblob"} & set(in_names)
    in_specs = tuple(
        PartitionSpec() if nm in rep_names else PartitionSpec("core")
        for nm in in_names
    ) + (PartitionSpec("core"),) * n_outs
    sharded = jax.jit(
        shard_map(_body, mesh=mesh, in_specs=in_specs,
                  out_specs=(PartitionSpec("core"),) * len(out_names),
                  check_rep=False),
        donate_argnums=tuple(range(n_params, n_params + n_outs)),
        keep_unused=True,
    )

    # Output buffers are pure scratch (the kernel fully overwrites out_d), so
    # they are chained across calls via donation: the first call ships zeros,
    # every later call re-donates the previous device-resident buffers. This
    # removes an 8 MB host->device upload (~80 ms of axon RPC) per warm call.
    state = {"outs": None}

    def submit(in_maps):
        """Upload fresh inputs, launch, return device output handles."""
        per_core = [[np.asarray(m[name]) for name in in_names] for m in in_maps]
        args_in = [
            per_core[0][i] if in_names[i] in rep_names else
            np.concatenate([per_core[c][i] for c in range(NC)], axis=0)
            for i in range(n_params)
        ]
        outs = state["outs"]
        if outs is None:
            outs = [np.zeros((NC * sh[0], *sh[1:]), dt) for sh, dt in zero_shapes]
        out_arrs = list(sharded(*args_in, *outs))
        state["outs"] = out_arrs
        jax.block_until_ready(out_arrs)
        return out_arrs

    def run(in_maps):
        out_arrs = submit(in_maps)
        return [
            {name: np.asarray(out_arrs[i]).reshape(NC, *out_avals[i].shape)[c]
             for i, name in enumerate(out_names)}
            for c in range(NC)
        ]

    run.submit = submit
    return run


def kernel(**inputs):
    if "nc" not in _CACHE:
        _CACHE["nc"] = build()
        _CACHE["run"] = _make_runner(_CACHE["nc"])
    in_maps = make_in_maps(inputs)
    results = _CACHE["run"](in_maps)
    out = np.concatenate([results[c]["out"] for c in range(NC)], axis=0)
    return out.reshape(B, N, H)


if __name__ == "__main__":
    rng = np.random.default_rng(0)
    fake = {"pos": rng.standard_normal((B, N, 3)).astype(np.float32)}
    for pfx in ("1", "2", "3"):
        d2 = 6 if pfx == "1" else 128
        fake[f"w{pfx}1"] = rng.standard_normal((d2, H)).astype(np.float32) * 0.2
        fake[f"w{pfx}2"] = rng.standard_normal((H, H)).astype(np.float32) * 0.12
        fake[f"w{pfx}3"] = rng.standard_normal((H, H)).astype(np.float32) * 0.12
        for j in ("1", "2", "3"):
            fake[f"b{pfx}{j}"] = np.zeros(H, np.float32)
    o = kernel(**fake)
    print("out", o.shape, o.dtype, float(np.abs(o).max()))
